# revision 25
# baseline (speedup 1.0000x reference)
"""Multi-head causal self-attention (B=2, S=2048, D=1024, H=16) on 8 TRN2 cores.

Sharding: core = b*4 + hg  (b in {0,1} batch, hg in {0..3} head-group of 4 heads).
Per core: project qT/kT (pair-packed [128, S], fp16) and v ([S, 64] blocks, fp16),
compute transposed scores S^T = K Q^T per head (k on partitions, two heads per
k-tile via tile_position), exp on ScalarE (both heads in one strided activation,
fp16 out), causal diag masking via one fused strided tensor_mul against a
duplicated upper-tri matrix, PV matmul with a ones-column PREPENDED to V (col 0
of each 128-wide head block, so row 0 of the accumulator is the softmax sum on
partition 0 — the DVE reciprocal mis-executes on HW with a partition-offset
input) and the block zero-padded to 128 columns so the compiler's fast-weight-
load path (NumWeights==128) keeps PV LDWEIGHTS off the critical path.
Normalization per chunk: copy the value rows out of PSUM, reciprocal straight
from the PSUM sums row, then (deferred one-per-kt into the next chunk) a
broadcast of rec and a fused multiply+cast per head. Pair-0 norms broadcast on
the idle GpSimd Q7 ring (PE is ~98% busy there); pair-1 norms broadcast on the
tensor engine (ones[1,64].T @ rec16 — PE has slack there and the 1.8us Q7
launches otherwise sit in pair-1's critical path and re-throttle the PE).
Output projection: per 128-row block both 512-wide halves cast into one
[128,1024] SBUF tile (casts split across DVE and ACT), then a single HWDGE DMA
on the Sync ring per block. Host sums the 4 per-batch partials and adds
(b_v @ w_o.T + b_o); b_k is dropped (softmax is invariant to per-query
constants); b_q is applied on-device.
Startup: inputs stream on the Sync HWDGE ring (cheap RTL descriptor generation;
the gpsimd SWDGE DIRECT2D path costs ~1us of Q7 per trigger and serialized the
old startup) in exact need order with wq split so the first projection chain
starts after ~400KB; the x tail columns + w_o ride the gpsimd ring in dead
time. While the first inputs stream, the PE runs warmup matmuls on a memset
scratch tile so the HAM activity window is full (2.4 GHz) when real work lands.
All matmul operands are fp16 (same PE rate as bf16, 8x the mantissa);
accumulation is fp32 in PSUM.
"""

import numpy as np
import ml_dtypes

import concourse.bass as bass
import concourse.mybir as mybir
import concourse.tile as tile
from concourse import bacc
from concourse.bass_utils import run_bass_kernel_spmd

B, S, D, H, DK = 2, 2048, 1024, 16, 64
N_CORES = 8
F32 = mybir.dt.float32
F16 = mybir.dt.float16
NPF16 = np.float16
AF = mybir.ActivationFunctionType

CHUNKS = [
    [(0, 512), (512, 512), (1024, 512), (1536, 512)],
    [(0, 512), (512, 512), (1024, 512), (1536, 512)],
]

NWARM = 5  # PE warmup matmuls (N=512) during the startup DMA wait


def _build(debug=False):
    nc = bacc.Bacc("TRN2", target_bir_lowering=False, debug=False,
                   num_devices=N_CORES)
    xT = nc.dram_tensor("xT", [D, S], F16, kind="ExternalInput").ap()
    wqT = nc.dram_tensor("wqT", [D, 256], F16, kind="ExternalInput").ap()
    wkT = nc.dram_tensor("wkT", [D, 256], F16, kind="ExternalInput").ap()
    wvT = nc.dram_tensor("wvT", [D, 256], F16, kind="ExternalInput").ap()
    woT = nc.dram_tensor("woT", [256, D], F16, kind="ExternalInput").ap()
    bq2 = nc.dram_tensor("bq2", [128, 2], F32, kind="ExternalInput").ap()
    tri2 = nc.dram_tensor("tri2", [128, 256], F16, kind="ExternalInput").ap()
    scr = nc.dram_tensor("scr", [2, 1024], F32, kind="Internal").ap()
    y = nc.dram_tensor("y", [S, D], F16, kind="ExternalOutput").ap()
    dbg = {}
    if debug:
        for nm, shp in [("qT", [128, 2, S]), ("kT", [128, 2, S]),
                        ("vv", [128, 16, 260]), ("oT", [128, 2, S])]:
            dbg[nm] = nc.dram_tensor(nm, shp, F16, kind="ExternalOutput").ap()

    NKT = S // 128   # k tiles
    # c-block views of the DRAM inputs: row (c*128+p) -> [p, c, w]
    xT_r = xT.rearrange("(c p) w -> p c w", p=128)
    wqT_r = wqT.rearrange("(c p) w -> p c w", p=128)
    wkT_r = wkT.rearrange("(c p) w -> p c w", p=128)
    wvT_r = wvT.rearrange("(c p) w -> p c w", p=128)

    with tile.TileContext(nc) as tc, \
            nc.allow_low_precision(reason="fp16 attention kernel"):
        with (
            tc.tile_pool(name="persist", bufs=1) as persist,
            tc.tile_pool(name="kqv", bufs=2) as kqv,
        ):
            qT_sb = [kqv.tile([128, S], F16, tag="qT", name=f"qT{p}") for p in range(2)]
            kT_sb = [kqv.tile([128, S], F16, tag="kT", name=f"kT{p}") for p in range(2)]
            v_sb = [persist.tile([128, 4 * 65], F16, tag=f"v{t}", name=f"v{t}")
                    for t in range(NKT)]
            outT_sb = [persist.tile([128, S], F16, tag=f"oT{p}", name=f"oTs{p}")
                       for p in range(2)]
            wo_sb = [persist.tile([128, D], F16, tag=f"wo{p}", name=f"wo{p}")
                     for p in range(2)]
            tri_sb = persist.tile([128, 256], F16, tag="tri")
            bq_sb = persist.tile([128, 2], F32, tag="bq")
            ones_sb = persist.tile([1, 64], F16, tag="ones1")
            tri_v = tri_sb.rearrange("p (c w) -> p c w", c=2)

            with (
                tc.tile_pool(name="xw", bufs=1) as xw,
                tc.tile_pool(name="ep", bufs=5) as ep,
                tc.tile_pool(name="rp", bufs=4) as rp,
            ):
                xt0_t = xw.tile([128, 8 * 512], F16, tag="x0", name="xt0")
                xt1_t = xw.tile([128, 8 * 1536], F16, tag="x1", name="xt1")
                wq_t = xw.tile([128, 8 * 256], F16, tag="wq", name="wqs")
                wk_t = xw.tile([128, 8 * 256], F16, tag="wk", name="wks")
                wv_t = xw.tile([128, 8 * 256], F16, tag="wv", name="wvs")
                warm_t = xw.tile([128, 512], F16, tag="warm", name="warm")
                xt0 = xt0_t.rearrange("p (c w) -> p c w", c=8)
                xt1 = xt1_t.rearrange("p (c w) -> p c w", c=8)
                wq_v = wq_t.rearrange("p (c w) -> p c w", c=8)
                wk_v = wk_t.rearrange("p (c w) -> p c w", c=8)
                wv_v = wv_t.rearrange("p (c w) -> p c w", c=8)

                # Inputs split across three DMA rings, each in its own need
                # order, so the front transfers parallelize: per-ring FIFO +
                # ~0.6us first-byte latency per DMA made one ring too serial.
                # sync (HWDGE): the first q-chain's x + wq, split for an early
                # start; scalar (HWDGE): k/v-side weights; gpsimd (SWDGE,
                # ~1us Q7 descriptor gen each, all in dead startup time): the
                # late-need x tail and w_o.
                nc.sync.dma_start(out=xt0[:, 0:2, :], in_=xT_r[:, 0:2, 0:512])
                nc.sync.dma_start(out=wq_v[:, 0:2, :], in_=wqT_r[:, 0:2, :])
                nc.sync.dma_start(out=xt0[:, 2:4, :], in_=xT_r[:, 2:4, 0:512])
                nc.sync.dma_start(out=wq_v[:, 2:8, :], in_=wqT_r[:, 2:8, :])
                nc.sync.dma_start(out=xt0[:, 4:8, :], in_=xT_r[:, 4:8, 0:512])
                nc.scalar.dma_start(out=bq_sb, in_=bq2)
                nc.scalar.dma_start(out=wk_v, in_=wkT_r)
                nc.scalar.dma_start(out=tri_sb, in_=tri2)
                nc.scalar.dma_start(out=wv_v, in_=wvT_r)
                nc.gpsimd.dma_start(out=xt1[:, :, 0:512], in_=xT_r[:, :, 512:1024])
                nc.gpsimd.dma_start(out=xt1[:, :, 512:1024], in_=xT_r[:, :, 1024:1536])
                nc.gpsimd.dma_start(out=xt1[:, :, 1024:1536], in_=xT_r[:, :, 1536:S])
                for p in range(2):
                    nc.gpsimd.dma_start(out=wo_sb[p], in_=woT[p * 128:(p + 1) * 128, :])

                nc.vector.memset(ones_sb, 1.0)
                nc.vector.memset(warm_t, 0.125)

                def xcols(c, a, b):
                    if b <= 512:
                        return xt0[:, c, a:b]
                    assert a >= 512
                    return xt1[:, c, a - 512:b - 512]

                def qk_chain(p, j, which, pool):
                    ps = pool.tile([128, 512], F32, tag="proj", name="ps")
                    w_v_ = wq_v if which == "q" else wk_v
                    for c in range(8):
                        nc.tensor.matmul(
                            ps, w_v_[:, c, p * 128:(p + 1) * 128],
                            xcols(c, j * 512, (j + 1) * 512),
                            start=(c == 0), stop=(c == 7))
                    if which == "q":
                        nc.vector.tensor_scalar_add(
                            qT_sb[p][:, j * 512:(j + 1) * 512], ps,
                            bq_sb[:, p:p + 1])
                    else:
                        nc.vector.tensor_copy(
                            kT_sb[p][:, j * 512:(j + 1) * 512], ps)

                def v_chain(t, pool):
                    ps_v = pool.tile([128, 256], F32, tag="proj", name="ps_v")
                    for c in range(8):
                        nc.tensor.matmul(
                            ps_v, xcols(c, t * 128, (t + 1) * 128), wv_v[:, c, :],
                            start=(c == 0), stop=(c == 7))
                    v_view = v_sb[t].rearrange("p (h w) -> p h w", w=65)
                    nc.vector.memset(v_view[:, :, 64:65], 1.0)
                    nc.vector.tensor_copy(
                        v_view[:, :, 0:64],
                        ps_v.rearrange("p (h w) -> p h w", w=64))

                norm_rest = []

                def emit_norm(p_, q0_, a_, b_, o_ps_, defer=True,
                              mm_bcast=False):
                    # copy values + the sums row out of PSUM for chunk columns
                    # [a, b); sums staged to partition 0 (PSUM reads must
                    # start partition-aligned, and the DVE reciprocal
                    # mis-executes on HW with a partition-offset input).
                    # The broadcast launches HERE (Q7 is idle; its ~1.8us
                    # latency hides across the chunk boundary) — only the
                    # final multiplies defer into the next chunk's slots. The
                    # staged last-chunk norms broadcast on the tensor engine
                    # instead (zero added latency before the output tail).
                    w_ = b_ - a_
                    rec = rp.tile([1, 1024], F32, tag="rec", name="recip")
                    sums = rp.tile([1, 1024], F32, tag="sums", name="sums")
                    ocs = []
                    for s in range(2):
                        oc = rp.tile([64, 512], F32, tag=f"oc{s}", name="o_cp")
                        nc.vector.tensor_copy(oc[:, 0:w_], o_ps_[s][0:64, a_:b_])
                        nc.vector.tensor_copy(
                            sums[:, s * 512:s * 512 + w_],
                            o_ps_[s][64:65, a_:b_])
                        ocs.append(oc)
                    for s in range(2):
                        nc.vector.reciprocal_approx_fast(
                            out=rec[:, s * 512:s * 512 + w_],
                            in_=sums[:, s * 512:s * 512 + w_])
                    if mm_bcast:
                        rec16 = rp.tile([1, 1024], F16, tag="rec16", name="r16")
                        nc.vector.tensor_copy(rec16, rec)
                        bc_list = []
                        for s in range(2):
                            bc_ps = fps.tile([128, 512], F32, tag="f", name="bcp")
                            nc.tensor.matmul(
                                bc_ps[0:64, :], ones_sb,
                                rec16[:, s * 512:(s + 1) * 512],
                                start=True, stop=True, skip_group_check=True)
                            bc_list.append(bc_ps[0:64, :])
                    else:
                        bcs = rp.tile([64, 1024], F32, tag="bc", name="bc")
                        nc.gpsimd.partition_broadcast(bcs, rec)
                        bc_list = [bcs[:, s * 512:(s + 1) * 512] for s in range(2)]
                    if defer:
                        norm_rest.append(
                            lambda: finish_norm(p_, q0_ + a_, w_, bc_list, ocs))
                    else:
                        finish_norm(p_, q0_ + a_, w_, bc_list, ocs)

                def finish_norm(p_, qa_, w_, bc_list, ocs):
                    for s in range(2):
                        nc.vector.tensor_mul(
                            outT_sb[p_][s * 64:(s + 1) * 64, qa_:qa_ + w_],
                            ocs[s][:, 0:w_],
                            bc_list[s][:, 0:w_])

                def emit_pair(p, fillers, stage=None):
                    # stage: (split_col, hookA, hookB) applied to the LAST
                    # chunk: columns [0, split) normalize right after their
                    # final PV (two k-tiles early), so most of the output
                    # projection tail overlaps the chunk's trailing k-tiles.
                    nchunks = len(CHUNKS[p])
                    for ci, (q0, qcw) in enumerate(CHUNKS[p]):
                        nkt = (q0 + qcw) // 128
                        last = stage is not None and ci == nchunks - 1
                        o_ps = [opp.tile([65, 512], F32, tag=f"o{s}", name=f"ops{s}")
                                for s in range(2)]
                        pend = None
                        for kt in range(nkt):
                            o = kt * 128 - q0
                            diag = o >= 0
                            lo = o if diag else 0
                            s_ab = sqp.tile([128, 1024], F32, tag="sq", name="s_ab")
                            s_v = s_ab.rearrange("p (c w) -> p c w", c=2)
                            for s in range(2):
                                nc.tensor.matmul(
                                    s_v[:, s, lo:qcw],
                                    kT_sb[p][s * 64:(s + 1) * 64,
                                             kt * 128:(kt + 1) * 128],
                                    qT_sb[p][s * 64:(s + 1) * 64,
                                             q0 + lo:q0 + qcw],
                                    start=True, stop=True,
                                    tile_position=(s * 64, 0),
                                    skip_group_check=True)
                            e_ab = ep.tile([128, 1024], F16, tag="e", name="e_ab")
                            e_v = e_ab.rearrange("p (c w) -> p c w", c=2)
                            nc.scalar.activation(
                                e_v[:, :, lo:qcw], s_v[:, :, lo:qcw],
                                AF.Exp, scale=0.125)
                            if diag:
                                nc.vector.tensor_mul(
                                    e_v[:, :, o:o + 128], e_v[:, :, o:o + 128],
                                    tri_v)
                            if norm_rest:
                                norm_rest.pop(0)()
                            if fillers is not None:
                                fillers(ci, kt, nkt)
                            if pend is not None:
                                _kt, _e, _lo = pend
                                for s in range(2):
                                    hb = 2 * p + s
                                    nc.tensor.matmul(
                                        o_ps[s][:, _lo:qcw],
                                        v_sb[_kt][:, hb * 65:(hb + 1) * 65],
                                        _e[:, s, _lo:qcw],
                                        start=(_kt == 0), stop=False,
                                        skip_group_check=True)
                            pend = (kt, e_v, lo)
                            if last and kt == nkt - 2:
                                # PV(kt-1) just issued; chunk columns
                                # [0, split) are final in PSUM
                                split, hookA, _ = stage
                                emit_norm(p, q0, 0, split, o_ps, defer=False,
                                          mm_bcast=True)
                                hookA()
                        _kt, _e, _lo = pend
                        for s in range(2):
                            hb = 2 * p + s
                            nc.tensor.matmul(
                                o_ps[s][:, _lo:qcw],
                                v_sb[_kt][:, hb * 65:(hb + 1) * 65],
                                _e[:, s, _lo:qcw],
                                start=False, stop=True,
                                skip_group_check=True)
                        if last:
                            split, _, hookB = stage
                            emit_norm(p, q0, split, qcw, o_ps, defer=False,
                                      mm_bcast=True)
                            hookB()
                        else:
                            emit_norm(p, q0, 0, qcw, o_ps)
                        yield ci
                    while norm_rest:
                        norm_rest.pop(0)()

                # ---- PE warmup + pair 0 front: j0 q/k chains ----
                with tc.tile_pool(name="ppsA", bufs=2, space="PSUM") as ppsA:
                    ps_q0 = ppsA.tile([128, 512], F32, tag="projA", name="ps_q0")
                    ps_k0 = ppsA.tile([128, 512], F32, tag="projA", name="ps_k0")
                    for i in range(NWARM):
                        nc.tensor.matmul(ps_q0, warm_t[:, 0:128], warm_t,
                                         start=True, stop=True,
                                         skip_group_check=True)
                    for c in range(8):
                        nc.tensor.matmul(
                            ps_q0, wq_v[:, c, 0:128], xt0[:, c, :],
                            start=(c == 0), stop=(c == 7))
                    for c in range(8):
                        nc.tensor.matmul(
                            ps_k0, wk_v[:, c, 0:128], xt0[:, c, :],
                            start=(c == 0), stop=(c == 7))
                    nc.vector.tensor_scalar_add(
                        qT_sb[0][:, 0:512], ps_q0, bq_sb[:, 0:1])
                    nc.vector.tensor_copy(kT_sb[0][:, 0:512], ps_k0)
                with (
                    tc.tile_pool(name="sq", bufs=2, space="PSUM") as sqp,
                    tc.tile_pool(name="ops", bufs=1, space="PSUM") as opp,
                ):
                    with tc.tile_pool(name="pps", bufs=2, space="PSUM") as pps:
                        fillers = []
                        for j in range(1, 4):
                            fillers.append(lambda j=j: qk_chain(0, j, "q", pps))
                            fillers.append(lambda j=j: qk_chain(0, j, "k", pps))
                            for t in range(4 * j, 4 * j + 4):
                                fillers.append(lambda t=t: v_chain(t, pps))
                        # chunk 0 gets v0-3 + j1 + v4-7, chunk 1 j2+v8-11,
                        # chunk 2 j3+v12-15, chunk 3 all 8 pair-1 projections
                        v03 = [lambda t=t: v_chain(t, pps) for t in range(4)]
                        sched = {0: v03 + fillers[0:6], 1: fillers[6:12],
                                 2: fillers[12:18]}
                        sched[3] = []
                        for j in range(4):
                            sched[3].append(lambda j=j: qk_chain(1, j, "q", pps))
                            sched[3].append(lambda j=j: qk_chain(1, j, "k", pps))
                        queues = [list(sched.get(ci, [])) for ci in range(4)]

                        def filler_pop(ci, kt, nkt):
                            q = queues[ci]
                            rem_slots = nkt - kt
                            while q and len(q) >= rem_slots:
                                q.pop(0)()
                            if q:
                                q.pop(0)()

                        for _ci in emit_pair(0, filler_pop):
                            while queues[_ci]:
                                queues[_ci].pop(0)()

                    if debug:
                        for p in range(2):
                            nc.sync.dma_start(out=dbg["qT"][:, p, :], in_=qT_sb[p])
                            nc.sync.dma_start(out=dbg["kT"][:, p, :], in_=kT_sb[p])
                        for t in range(NKT):
                            nc.sync.dma_start(out=dbg["vv"][:, t, :], in_=v_sb[t])

                    # ---- pair 1: output projection as fillers ----
                    with tc.tile_pool(name="fps", bufs=2, space="PSUM") as fps:
                        with tc.tile_pool(name="fsb", bufs=4) as fsb:
                            c_alt = [0]
                            fsb_tiles = {}

                            def c_unit(qt, oc_i, vec=None):
                                f_ps = fps.tile([128, 512], F32, tag="f", name="f_ps")
                                for p in range(2):
                                    nc.tensor.matmul(
                                        f_ps, outT_sb[p][:, qt * 128:(qt + 1) * 128],
                                        wo_sb[p][:, oc_i * 512:(oc_i + 1) * 512],
                                        start=(p == 0), stop=(p == 1))
                                if qt not in fsb_tiles:
                                    fsb_tiles[qt] = fsb.tile(
                                        [128, 1024], F16, tag="f", name=f"fsb{qt}")
                                half = fsb_tiles[qt][:, oc_i * 512:(oc_i + 1) * 512]
                                c_alt[0] ^= 1
                                use_vec = vec if vec is not None else c_alt[0]
                                if use_vec:
                                    nc.vector.tensor_copy(half, f_ps)
                                else:
                                    nc.scalar.activation(half, f_ps, AF.Identity)
                                if oc_i == 1:
                                    # one HWDGE DMA per 128-row output block
                                    nc.sync.dma_start(
                                        out=y[qt * 128:(qt + 1) * 128, :],
                                        in_=fsb_tiles.pop(qt))

                            NCH = len(CHUNKS[1])
                            cqueues = [[] for _ in range(NCH)]

                            def c_pop(ci, kt, nkt):
                                q = cqueues[ci]
                                rem_slots = nkt - kt
                                while q and len(q) >= rem_slots:
                                    q.pop(0)()
                                if q:
                                    q.pop(0)()

                            qt_ranges = [(0, 4), (4, 8), (8, 12)]

                            def tail_hook_a():
                                # qt 12-13 project while k-tiles 14/15 finish;
                                # casts split across DVE and ACT (both idle)
                                for qt in (12, 13):
                                    c_unit(qt, 0, vec=True)
                                    c_unit(qt, 1, vec=False)

                            def tail_hook_b():
                                for qt in (14, 15):
                                    c_unit(qt, 0, vec=True)
                                    c_unit(qt, 1, vec=False)

                            for ci in emit_pair(1, c_pop,
                                                stage=(256, tail_hook_a,
                                                       tail_hook_b)):
                                if ci < len(qt_ranges):
                                    a, b = qt_ranges[ci]
                                    units = []
                                    for qt in range(a, b):
                                        for oc_i in range(2):
                                            units.append(
                                                lambda qt=qt, oc_i=oc_i:
                                                c_unit(qt, oc_i))
                                    cqueues[ci + 1].extend(units)
                            for q in cqueues:
                                while q:
                                    q.pop(0)()

            if debug:
                for p in range(2):
                    nc.sync.dma_start(out=dbg["oT"][:, p, :], in_=outT_sb[p])

    nc.compile()
    return nc


_cached = {}


def _get_nc(debug=False):
    key = bool(debug)
    if key not in _cached:
        _cached[key] = _build(debug)
    return _cached[key]


def _prep_inputs(x, w_q, b_q, w_k, w_v):
    tri = np.triu(np.ones((128, 128), np.float32)).astype(NPF16)
    tri2 = np.concatenate([tri, tri], axis=1)
    wqT_f = np.ascontiguousarray(w_q.T).astype(NPF16)
    wkT_f = np.ascontiguousarray(w_k.T).astype(NPF16)
    wvT_f = np.ascontiguousarray(w_v.T).astype(NPF16)
    in_maps = []
    for core in range(N_CORES):
        b, hg = divmod(core, 4)
        cs = slice(hg * 256, (hg + 1) * 256)
        in_maps.append({
            "xT": np.ascontiguousarray(x[b].T).astype(NPF16),
            "wqT": np.ascontiguousarray(wqT_f[:, cs]),
            "wkT": np.ascontiguousarray(wkT_f[:, cs]),
            "wvT": np.ascontiguousarray(wvT_f[:, cs]),
            "bq2": np.ascontiguousarray(
                b_q[hg * 256:(hg + 1) * 256].reshape(2, 128).T.astype(np.float32)),
            "tri2": tri2,
        })
    return in_maps


def _numpy_reference(x, attention_mask, w_q, b_q, w_k, b_k, w_v, b_v, w_o, b_o):
    x = x.astype(np.float64)
    q = (x @ w_q.T + b_q).reshape(B, S, H, DK).transpose(0, 2, 1, 3)
    k = (x @ w_k.T + b_k).reshape(B, S, H, DK).transpose(0, 2, 1, 3)
    v = (x @ w_v.T + b_v).reshape(B, S, H, DK).transpose(0, 2, 1, 3)
    scores = np.einsum("bhqd,bhkd->bhqk", q, k, optimize=True) / np.sqrt(DK)
    causal = np.tril(np.ones((S, S), bool))
    mask = causal[None, None] & (attention_mask[:, None, None, :] != 0)
    scores = np.where(mask, scores, -np.inf)
    scores -= scores.max(-1, keepdims=True)
    e = np.exp(scores)
    attn = e / e.sum(-1, keepdims=True)
    out = np.einsum("bhqk,bhkd->bhqd", attn, v, optimize=True)
    out = out.transpose(0, 2, 1, 3).reshape(B, S, D)
    return (out @ w_o.T + b_o).astype(np.float32)


def kernel(x, attention_mask, w_q, b_q, w_k, b_k, w_v, b_v, w_o, b_o,
           _debug=False, _trace=False):
    x = np.asarray(x, np.float32)
    attention_mask = np.asarray(attention_mask)
    if not np.all(attention_mask != 0):
        return _numpy_reference(np.asarray(x), np.asarray(attention_mask),
                                *[np.asarray(a) for a in
                                  (w_q, b_q, w_k, b_k, w_v, b_v, w_o, b_o)])
    w_q, w_k, w_v, w_o = [np.asarray(w, np.float32) for w in (w_q, w_k, w_v, w_o)]
    b_q, b_k, b_v, b_o = [np.asarray(b, np.float32) for b in (b_q, b_k, b_v, b_o)]

    nc = _get_nc(_debug)
    in_maps = _prep_inputs(x, w_q, b_q, w_k, w_v)
    woT_f = np.ascontiguousarray(w_o.T).astype(NPF16)
    for core in range(N_CORES):
        hg = core % 4
        in_maps[core]["woT"] = np.ascontiguousarray(
            woT_f[hg * 256:(hg + 1) * 256, :])

    res = run_bass_kernel_spmd(nc, in_maps, list(range(N_CORES)), trace=_trace)
    const_row = (b_v @ w_o.T + b_o).astype(np.float32)
    y = np.zeros((B, S, D), np.float32)
    for core in range(N_CORES):
        b = core // 4
        y[b] += res.results[core]["y"].astype(np.float32)
    y += const_row
    if _debug or _trace:
        return y, res
    return y


# revision 28
# speedup vs baseline: 1.0236x; 1.0236x over previous
"""Multi-head causal self-attention (B=2, S=2048, D=1024, H=16) on 8 TRN2 cores.

Sharding: core = b*4 + hg  (b in {0,1} batch, hg in {0..3} head-group of 4 heads).
Per core: project qT/kT (pair-packed [128, S], fp16) and v ([S, 64] blocks, fp16),
compute transposed scores S^T = K Q^T per head (k on partitions, two heads per
k-tile via tile_position), exp on ScalarE (both heads in one strided activation,
fp16 out), causal diag masking via one fused strided tensor_mul against a
duplicated upper-tri matrix, PV matmul with a ones-column PREPENDED to V (col 0
of each 128-wide head block, so row 0 of the accumulator is the softmax sum on
partition 0 — the DVE reciprocal mis-executes on HW with a partition-offset
input) and the block zero-padded to 128 columns so the compiler's fast-weight-
load path (NumWeights==128) keeps PV LDWEIGHTS off the critical path.
Normalization per chunk: copy the value rows out of PSUM, reciprocal straight
from the PSUM sums row, then (deferred one-per-kt into the next chunk) a
broadcast of rec and a fused multiply+cast per head. Pair-0 norms broadcast on
the idle GpSimd Q7 ring (PE is ~98% busy there); pair-1 norms broadcast on the
tensor engine (ones[1,64].T @ rec16 — PE has slack there and the 1.8us Q7
launches otherwise sit in pair-1's critical path and re-throttle the PE).
Output projection: per 128-row block both 512-wide halves cast into one
[128,1024] SBUF tile (casts split across DVE and ACT), then a single HWDGE DMA
on the Sync ring per block. Host sums the 4 per-batch partials and adds
(b_v @ w_o.T + b_o); b_k is dropped (softmax is invariant to per-query
constants); b_q is applied on-device.
Startup: inputs stream on the Sync HWDGE ring (cheap RTL descriptor generation;
the gpsimd SWDGE DIRECT2D path costs ~1us of Q7 per trigger and serialized the
old startup) in exact need order with wq split so the first projection chain
starts after ~400KB; the x tail columns + w_o ride the gpsimd ring in dead
time. While the first inputs stream, the PE runs warmup matmuls on a memset
scratch tile so the HAM activity window is full (2.4 GHz) when real work lands.
All matmul operands are fp16 (same PE rate as bf16, 8x the mantissa);
accumulation is fp32 in PSUM.
"""

import numpy as np
import ml_dtypes

import concourse.bass as bass
import concourse.mybir as mybir
import concourse.tile as tile
from concourse import bacc
from concourse.bass_utils import run_bass_kernel_spmd

B, S, D, H, DK = 2, 2048, 1024, 16, 64
N_CORES = 8
F32 = mybir.dt.float32
F16 = mybir.dt.float16
NPF16 = np.float16
AF = mybir.ActivationFunctionType

CHUNKS = [
    [(0, 512), (512, 512), (1024, 512), (1536, 512)],
    [(0, 512), (512, 512), (1024, 512), (1536, 512)],
]

NWARM = 5  # PE warmup matmuls (N=512) during the startup DMA wait


def _build(debug=False):
    nc = bacc.Bacc("TRN2", target_bir_lowering=False, debug=False,
                   num_devices=N_CORES)
    # inputs are host-packed into the exact SBUF tile layouts ("p (c w)"),
    # so every DMA is a partition-contiguous slab (max SDMA efficiency)
    x0p = nc.dram_tensor("x0p", [128, 4096], F16, kind="ExternalInput").ap()
    x1ap = nc.dram_tensor("x1ap", [128, 4096], F16, kind="ExternalInput").ap()
    x1bp = nc.dram_tensor("x1bp", [128, 4096], F16, kind="ExternalInput").ap()
    x1cp = nc.dram_tensor("x1cp", [128, 4096], F16, kind="ExternalInput").ap()
    wqp = nc.dram_tensor("wqp", [128, 2048], F16, kind="ExternalInput").ap()
    wkp = nc.dram_tensor("wkp", [128, 2048], F16, kind="ExternalInput").ap()
    wvp = nc.dram_tensor("wvp", [128, 2048], F16, kind="ExternalInput").ap()
    woT = nc.dram_tensor("woT", [256, D], F16, kind="ExternalInput").ap()
    bq2 = nc.dram_tensor("bq2", [128, 2], F32, kind="ExternalInput").ap()
    tri2 = nc.dram_tensor("tri2", [128, 256], F16, kind="ExternalInput").ap()
    scr = nc.dram_tensor("scr", [2, 1024], F32, kind="Internal").ap()
    y = nc.dram_tensor("y", [S, D], F16, kind="ExternalOutput").ap()
    dbg = {}
    if debug:
        for nm, shp in [("qT", [128, 2, S]), ("kT", [128, 2, S]),
                        ("vv", [128, 16, 260]), ("oT", [128, 2, S])]:
            dbg[nm] = nc.dram_tensor(nm, shp, F16, kind="ExternalOutput").ap()

    NKT = S // 128   # k tiles

    with tile.TileContext(nc) as tc, \
            nc.allow_low_precision(reason="fp16 attention kernel"):
        with (
            tc.tile_pool(name="persist", bufs=1) as persist,
            tc.tile_pool(name="kqv", bufs=2) as kqv,
        ):
            qT_sb = [kqv.tile([128, S], F16, tag="qT", name=f"qT{p}") for p in range(2)]
            kT_sb = [kqv.tile([128, S], F16, tag="kT", name=f"kT{p}") for p in range(2)]
            v_sb = [persist.tile([128, 4 * 65], F16, tag=f"v{t}", name=f"v{t}")
                    for t in range(NKT)]
            outT_sb = [persist.tile([128, S], F16, tag=f"oT{p}", name=f"oTs{p}")
                       for p in range(2)]
            wo_sb = [persist.tile([128, D], F16, tag=f"wo{p}", name=f"wo{p}")
                     for p in range(2)]
            tri_sb = persist.tile([128, 256], F16, tag="tri")
            bq_sb = persist.tile([128, 2], F32, tag="bq")
            ones_sb = persist.tile([1, 64], F16, tag="ones1")
            tri_v = tri_sb.rearrange("p (c w) -> p c w", c=2)

            with (
                tc.tile_pool(name="xw", bufs=1) as xw,
                tc.tile_pool(name="ep", bufs=5) as ep,
                tc.tile_pool(name="rp", bufs=4) as rp,
            ):
                xt0_t = xw.tile([128, 8 * 512], F16, tag="x0", name="xt0")
                xt1a_t = xw.tile([128, 8 * 512], F16, tag="x1a", name="xt1a")
                xt1b_t = xw.tile([128, 8 * 512], F16, tag="x1b", name="xt1b")
                xt1c_t = xw.tile([128, 8 * 512], F16, tag="x1c", name="xt1c")
                wq_t = xw.tile([128, 8 * 256], F16, tag="wq", name="wqs")
                wk_t = xw.tile([128, 8 * 256], F16, tag="wk", name="wks")
                wv_t = xw.tile([128, 8 * 256], F16, tag="wv", name="wvs")
                warm_t = xw.tile([128, 512], F16, tag="warm", name="warm")
                xt0 = xt0_t.rearrange("p (c w) -> p c w", c=8)
                xt1v = [t.rearrange("p (c w) -> p c w", c=8)
                        for t in (xt1a_t, xt1b_t, xt1c_t)]
                wq_v = wq_t.rearrange("p (c w) -> p c w", c=8)
                wk_v = wk_t.rearrange("p (c w) -> p c w", c=8)
                wv_v = wv_t.rearrange("p (c w) -> p c w", c=8)

                # Two HWDGE rings in parallel (each sustains only ~half the
                # HBM wire), both in need order: x-side on sync, weight-side
                # on scalar; tri alone on the gpsimd SWDGE ring. Every
                # transfer is a contiguous slab of a host-packed tensor.
                nc.sync.dma_start(out=xt0_t[:, 0:1024], in_=x0p[:, 0:1024])
                nc.sync.dma_start(out=xt0_t[:, 1024:2048], in_=x0p[:, 1024:2048])
                nc.sync.dma_start(out=xt0_t[:, 2048:4096], in_=x0p[:, 2048:4096])
                nc.sync.dma_start(out=xt1a_t, in_=x1ap)
                nc.sync.dma_start(out=xt1b_t, in_=x1bp)
                nc.sync.dma_start(out=xt1c_t, in_=x1cp)
                nc.scalar.dma_start(out=bq_sb, in_=bq2)
                nc.scalar.dma_start(out=wq_t[:, 0:512], in_=wqp[:, 0:512])
                nc.scalar.dma_start(out=wq_t[:, 512:2048], in_=wqp[:, 512:2048])
                nc.scalar.dma_start(out=wk_t, in_=wkp)
                nc.scalar.dma_start(out=wv_t, in_=wvp)
                for p in range(2):
                    nc.scalar.dma_start(out=wo_sb[p], in_=woT[p * 128:(p + 1) * 128, :])
                nc.gpsimd.dma_start(out=tri_sb, in_=tri2)

                nc.vector.memset(ones_sb, 1.0)
                nc.vector.memset(warm_t, 0.125)

                def xcols(c, a, b):
                    blk = a // 512
                    assert b <= (blk + 1) * 512
                    if blk == 0:
                        return xt0[:, c, a:b]
                    t = xt1v[blk - 1]
                    return t[:, c, a - blk * 512:b - blk * 512]

                def qk_chain(p, j, which, pool, tag="proj"):
                    ps = pool.tile([128, 512], F32, tag=tag, name="ps")
                    w_v_ = wq_v if which == "q" else wk_v
                    for c in range(8):
                        nc.tensor.matmul(
                            ps, w_v_[:, c, p * 128:(p + 1) * 128],
                            xcols(c, j * 512, (j + 1) * 512),
                            start=(c == 0), stop=(c == 7))
                    if which == "q":
                        nc.vector.tensor_scalar_add(
                            qT_sb[p][:, j * 512:(j + 1) * 512], ps,
                            bq_sb[:, p:p + 1])
                    else:
                        nc.vector.tensor_copy(
                            kT_sb[p][:, j * 512:(j + 1) * 512], ps)

                def v_chain(t, pool):
                    ps_v = pool.tile([128, 256], F32, tag="proj", name="ps_v")
                    for c in range(8):
                        nc.tensor.matmul(
                            ps_v, xcols(c, t * 128, (t + 1) * 128), wv_v[:, c, :],
                            start=(c == 0), stop=(c == 7))
                    v_view = v_sb[t].rearrange("p (h w) -> p h w", w=65)
                    nc.vector.memset(v_view[:, :, 64:65], 1.0)
                    nc.vector.tensor_copy(
                        v_view[:, :, 0:64],
                        ps_v.rearrange("p (h w) -> p h w", w=64))

                norm_rest = []

                def emit_norm(p_, q0_, a_, b_, o_ps_, defer=True,
                              mm_bcast=False):
                    # copy values + the sums row out of PSUM for chunk columns
                    # [a, b); sums staged to partition 0 (PSUM reads must
                    # start partition-aligned, and the DVE reciprocal
                    # mis-executes on HW with a partition-offset input).
                    # The broadcast launches HERE (Q7 is idle; its ~1.8us
                    # latency hides across the chunk boundary) — only the
                    # final multiplies defer into the next chunk's slots. The
                    # staged last-chunk norms broadcast on the tensor engine
                    # instead (zero added latency before the output tail).
                    w_ = b_ - a_
                    rec = rp.tile([1, 1024], F32, tag="rec", name="recip")
                    sums = rp.tile([1, 1024], F32, tag="sums", name="sums")
                    ocs = []
                    for s in range(2):
                        oc = rp.tile([64, 512], F32, tag=f"oc{s}", name="o_cp")
                        nc.vector.tensor_copy(oc[:, 0:w_], o_ps_[s][0:64, a_:b_])
                        nc.vector.tensor_copy(
                            sums[:, s * 512:s * 512 + w_],
                            o_ps_[s][64:65, a_:b_])
                        ocs.append(oc)
                    for s in range(2):
                        nc.vector.reciprocal_approx_fast(
                            out=rec[:, s * 512:s * 512 + w_],
                            in_=sums[:, s * 512:s * 512 + w_])
                    if mm_bcast:
                        rec16 = rp.tile([1, 1024], F16, tag="rec16", name="r16")
                        nc.vector.tensor_copy(rec16, rec)
                        bc_list = []
                        for s in range(2):
                            bc_ps = fps.tile([128, 512], F32, tag="f", name="bcp")
                            nc.tensor.matmul(
                                bc_ps[0:64, :], ones_sb,
                                rec16[:, s * 512:(s + 1) * 512],
                                start=True, stop=True, skip_group_check=True)
                            bc_list.append(bc_ps[0:64, :])
                    else:
                        bcs = rp.tile([64, 1024], F32, tag="bc", name="bc")
                        nc.gpsimd.partition_broadcast(bcs, rec)
                        bc_list = [bcs[:, s * 512:(s + 1) * 512] for s in range(2)]
                    if defer:
                        # deferred multiplies run on the mostly-idle GpSimd
                        # ring (all-SBUF operands) to unload the DVE, which is
                        # the #2 engine in pair 1
                        norm_rest.append(
                            lambda: finish_norm(p_, q0_ + a_, w_, bc_list, ocs,
                                                eng=nc.gpsimd))
                    else:
                        finish_norm(p_, q0_ + a_, w_, bc_list, ocs,
                                    eng=nc.vector)

                def finish_norm(p_, qa_, w_, bc_list, ocs, eng):
                    for s in range(2):
                        eng.tensor_mul(
                            outT_sb[p_][s * 64:(s + 1) * 64, qa_:qa_ + w_],
                            ocs[s][:, 0:w_],
                            bc_list[s][:, 0:w_])

                def emit_pair(p, fillers, stage=None):
                    # stage: (split_col, hookA, hookB) applied to the LAST
                    # chunk: columns [0, split) normalize right after their
                    # final PV (two k-tiles early), so most of the output
                    # projection tail overlaps the chunk's trailing k-tiles.
                    nchunks = len(CHUNKS[p])
                    for ci, (q0, qcw) in enumerate(CHUNKS[p]):
                        nkt = (q0 + qcw) // 128
                        last = stage is not None and ci == nchunks - 1
                        o_ps = [opp.tile([65, 512], F32, tag=f"o{s}", name=f"ops{s}")
                                for s in range(2)]
                        pend = None
                        for kt in range(nkt):
                            o = kt * 128 - q0
                            diag = o >= 0
                            lo = o if diag else 0
                            s_ab = sqp.tile([128, 1024], F32, tag="sq", name="s_ab")
                            s_v = s_ab.rearrange("p (c w) -> p c w", c=2)
                            for s in range(2):
                                nc.tensor.matmul(
                                    s_v[:, s, lo:qcw],
                                    kT_sb[p][s * 64:(s + 1) * 64,
                                             kt * 128:(kt + 1) * 128],
                                    qT_sb[p][s * 64:(s + 1) * 64,
                                             q0 + lo:q0 + qcw],
                                    start=True, stop=True,
                                    tile_position=(s * 64, 0),
                                    skip_group_check=True)
                            e_ab = ep.tile([128, 1024], F16, tag="e", name="e_ab")
                            e_v = e_ab.rearrange("p (c w) -> p c w", c=2)
                            nc.scalar.activation(
                                e_v[:, :, lo:qcw], s_v[:, :, lo:qcw],
                                AF.Exp, scale=0.125)
                            if diag:
                                nc.vector.tensor_mul(
                                    e_v[:, :, o:o + 128], e_v[:, :, o:o + 128],
                                    tri_v)
                            if norm_rest:
                                norm_rest.pop(0)()
                            if fillers is not None:
                                fillers(ci, kt, nkt)
                            if pend is not None:
                                _kt, _e, _lo = pend
                                for s in range(2):
                                    hb = 2 * p + s
                                    nc.tensor.matmul(
                                        o_ps[s][:, _lo:qcw],
                                        v_sb[_kt][:, hb * 65:(hb + 1) * 65],
                                        _e[:, s, _lo:qcw],
                                        start=(_kt == 0), stop=False,
                                        skip_group_check=True)
                            pend = (kt, e_v, lo)
                            if last and kt == nkt - 2:
                                # PV(kt-1) just issued; chunk columns
                                # [0, split) are final in PSUM
                                split, hookA, _ = stage
                                emit_norm(p, q0, 0, split, o_ps, defer=False,
                                          mm_bcast=True)
                                hookA()
                        _kt, _e, _lo = pend
                        for s in range(2):
                            hb = 2 * p + s
                            nc.tensor.matmul(
                                o_ps[s][:, _lo:qcw],
                                v_sb[_kt][:, hb * 65:(hb + 1) * 65],
                                _e[:, s, _lo:qcw],
                                start=False, stop=True,
                                skip_group_check=True)
                        if last:
                            split, _, hookB = stage
                            emit_norm(p, q0, split, qcw, o_ps, defer=False,
                                      mm_bcast=True)
                            hookB()
                        else:
                            emit_norm(p, q0, 0, qcw, o_ps)
                        yield ci
                    while norm_rest:
                        norm_rest.pop(0)()

                # ---- PE warmup + pair 0 front: j0 q/k chains ----
                with tc.tile_pool(name="ppsA", bufs=2, space="PSUM") as ppsA:
                    ps_q0 = ppsA.tile([128, 512], F32, tag="projA", name="ps_q0")
                    ps_k0 = ppsA.tile([128, 512], F32, tag="projA", name="ps_k0")
                    for i in range(NWARM):
                        nc.tensor.matmul(ps_q0, warm_t[:, 0:128], warm_t,
                                         start=True, stop=True,
                                         skip_group_check=True)
                    for c in range(8):
                        nc.tensor.matmul(
                            ps_q0, wq_v[:, c, 0:128], xt0[:, c, :],
                            start=(c == 0), stop=(c == 7))
                    for c in range(8):
                        nc.tensor.matmul(
                            ps_k0, wk_v[:, c, 0:128], xt0[:, c, :],
                            start=(c == 0), stop=(c == 7))
                    nc.vector.tensor_scalar_add(
                        qT_sb[0][:, 0:512], ps_q0, bq_sb[:, 0:1])
                    nc.vector.tensor_copy(kT_sb[0][:, 0:512], ps_k0)
                with (
                    tc.tile_pool(name="sq", bufs=2, space="PSUM") as sqp,
                    tc.tile_pool(name="ops", bufs=1, space="PSUM") as opp,
                ):
                    with tc.tile_pool(name="pps", bufs=2, space="PSUM") as pps:
                        # pair-0 fillers, placed to match DMA arrivals:
                        # chunk 0: v0-1 + pair-0 j1 (x block 1 lands mid-chunk)
                        # chunk 1: v2-7 + pair-0 j2
                        # chunk 2: v8-11 + pair-0 j3
                        # chunk 3: v12-15 + pair-1 j0 (pair-1 j1-j3 moved into
                        #          pair-1's own slots — pair 0 runs PE-
                        #          saturated while pair 1 has idle slots)
                        def qkf(p_, j_):
                            return [lambda: qk_chain(p_, j_, "q", pps),
                                    lambda: qk_chain(p_, j_, "k", pps)]

                        def vf(a_, b_):
                            return [lambda t=t: v_chain(t, pps)
                                    for t in range(a_, b_)]

                        # v(t) must be emitted in or before the chunk whose PV
                        # consumes it (chunk ci's PV covers k-tiles < 4*(ci+1))
                        sched = {0: vf(0, 4) + qkf(0, 1),
                                 1: vf(4, 8) + qkf(0, 2),
                                 2: vf(8, 12) + qkf(0, 3),
                                 3: vf(12, 16) + qkf(1, 0)}
                        queues = [list(sched.get(ci, [])) for ci in range(4)]

                        def filler_pop(ci, kt, nkt):
                            q = queues[ci]
                            rem_slots = nkt - kt
                            while q and len(q) >= rem_slots:
                                q.pop(0)()
                            if q:
                                q.pop(0)()

                        for _ci in emit_pair(0, filler_pop):
                            while queues[_ci]:
                                queues[_ci].pop(0)()

                    if debug:
                        for p in range(2):
                            nc.sync.dma_start(out=dbg["qT"][:, p, :], in_=qT_sb[p])
                            nc.sync.dma_start(out=dbg["kT"][:, p, :], in_=kT_sb[p])
                        for t in range(NKT):
                            nc.sync.dma_start(out=dbg["vv"][:, t, :], in_=v_sb[t])

                    # ---- pair 1: output projection as fillers ----
                    with tc.tile_pool(name="fps", bufs=2, space="PSUM") as fps:
                        with tc.tile_pool(name="fsb", bufs=4) as fsb:
                            c_alt = [0]
                            fsb_tiles = {}

                            def c_unit(qt, oc_i, vec=None):
                                f_ps = fps.tile([128, 512], F32, tag="f", name="f_ps")
                                for p in range(2):
                                    nc.tensor.matmul(
                                        f_ps, outT_sb[p][:, qt * 128:(qt + 1) * 128],
                                        wo_sb[p][:, oc_i * 512:(oc_i + 1) * 512],
                                        start=(p == 0), stop=(p == 1))
                                if qt not in fsb_tiles:
                                    fsb_tiles[qt] = fsb.tile(
                                        [128, 1024], F16, tag="f", name=f"fsb{qt}")
                                half = fsb_tiles[qt][:, oc_i * 512:(oc_i + 1) * 512]
                                c_alt[0] ^= 1
                                use_vec = vec if vec is not None else c_alt[0]
                                if use_vec:
                                    nc.vector.tensor_copy(half, f_ps)
                                else:
                                    nc.scalar.activation(half, f_ps, AF.Identity)
                                if oc_i == 1:
                                    # one HWDGE DMA per 128-row output block,
                                    # alternating the two HWDGE rings
                                    eng = nc.sync if qt % 2 == 0 else nc.scalar
                                    eng.dma_start(
                                        out=y[qt * 128:(qt + 1) * 128, :],
                                        in_=fsb_tiles.pop(qt))

                            NCH = len(CHUNKS[1])
                            # pair-1's own j1-j3 projection chains run as
                            # early-chunk fillers here (chunk ci's scores only
                            # need q/kT cols up to (ci+1)*512, so j(ci+1)
                            # finishing inside chunk ci is in time)
                            cqueues = [[] for _ in range(NCH)]
                            for ci in range(3):
                                if ci + 1 <= 3:
                                    cqueues[ci] += [
                                        lambda j=ci + 1: qk_chain(1, j, "q", fps, tag="f"),
                                        lambda j=ci + 1: qk_chain(1, j, "k", fps, tag="f")]

                            def c_pop(ci, kt, nkt):
                                q = cqueues[ci]
                                rem_slots = nkt - kt
                                while q and len(q) >= rem_slots:
                                    q.pop(0)()
                                if q:
                                    q.pop(0)()

                            qt_ranges = [(0, 4), (4, 8), (8, 12)]

                            def tail_hook_a():
                                # qt 12-13 project while k-tiles 14/15 finish;
                                # all casts on DVE — a scalar-engine cast here
                                # would queue behind the last exps (FIFO)
                                for qt in (12, 13):
                                    c_unit(qt, 0, vec=True)
                                    c_unit(qt, 1, vec=True)

                            def tail_hook_b():
                                for qt in (14, 15):
                                    c_unit(qt, 0, vec=True)
                                    c_unit(qt, 1, vec=True)

                            for ci in emit_pair(1, c_pop,
                                                stage=(256, tail_hook_a,
                                                       tail_hook_b)):
                                if ci < len(qt_ranges):
                                    a, b = qt_ranges[ci]
                                    units = []
                                    for qt in range(a, b):
                                        for oc_i in range(2):
                                            units.append(
                                                lambda qt=qt, oc_i=oc_i:
                                                c_unit(qt, oc_i))
                                    cqueues[ci + 1].extend(units)
                            for q in cqueues:
                                while q:
                                    q.pop(0)()

            if debug:
                for p in range(2):
                    nc.sync.dma_start(out=dbg["oT"][:, p, :], in_=outT_sb[p])

    nc.compile()
    return nc


_cached = {}


def _get_nc(debug=False):
    key = bool(debug)
    if key not in _cached:
        _cached[key] = _build(debug)
    return _cached[key]


def _pack_pcw(a):
    # [(c p), w] -> [p, (c w)]  (the SBUF tile layout)
    cp, w = a.shape
    c = cp // 128
    return np.ascontiguousarray(
        a.reshape(c, 128, w).transpose(1, 0, 2).reshape(128, c * w))


def _prep_inputs(x, w_q, b_q, w_k, w_v):
    tri = np.triu(np.ones((128, 128), np.float32)).astype(NPF16)
    tri2 = np.concatenate([tri, tri], axis=1)
    wqT_f = np.ascontiguousarray(w_q.T).astype(NPF16)
    wkT_f = np.ascontiguousarray(w_k.T).astype(NPF16)
    wvT_f = np.ascontiguousarray(w_v.T).astype(NPF16)
    in_maps = []
    for core in range(N_CORES):
        b, hg = divmod(core, 4)
        cs = slice(hg * 256, (hg + 1) * 256)
        xm = x[b].T.astype(NPF16)  # [D, S]
        in_maps.append({
            "x0p": _pack_pcw(xm[:, 0:512]),
            "x1ap": _pack_pcw(xm[:, 512:1024]),
            "x1bp": _pack_pcw(xm[:, 1024:1536]),
            "x1cp": _pack_pcw(xm[:, 1536:2048]),
            "wqp": _pack_pcw(wqT_f[:, cs]),
            "wkp": _pack_pcw(wkT_f[:, cs]),
            "wvp": _pack_pcw(wvT_f[:, cs]),
            "bq2": np.ascontiguousarray(
                b_q[hg * 256:(hg + 1) * 256].reshape(2, 128).T.astype(np.float32)),
            "tri2": tri2,
        })
    return in_maps


def _numpy_reference(x, attention_mask, w_q, b_q, w_k, b_k, w_v, b_v, w_o, b_o):
    x = x.astype(np.float64)
    q = (x @ w_q.T + b_q).reshape(B, S, H, DK).transpose(0, 2, 1, 3)
    k = (x @ w_k.T + b_k).reshape(B, S, H, DK).transpose(0, 2, 1, 3)
    v = (x @ w_v.T + b_v).reshape(B, S, H, DK).transpose(0, 2, 1, 3)
    scores = np.einsum("bhqd,bhkd->bhqk", q, k, optimize=True) / np.sqrt(DK)
    causal = np.tril(np.ones((S, S), bool))
    mask = causal[None, None] & (attention_mask[:, None, None, :] != 0)
    scores = np.where(mask, scores, -np.inf)
    scores -= scores.max(-1, keepdims=True)
    e = np.exp(scores)
    attn = e / e.sum(-1, keepdims=True)
    out = np.einsum("bhqk,bhkd->bhqd", attn, v, optimize=True)
    out = out.transpose(0, 2, 1, 3).reshape(B, S, D)
    return (out @ w_o.T + b_o).astype(np.float32)


def kernel(x, attention_mask, w_q, b_q, w_k, b_k, w_v, b_v, w_o, b_o,
           _debug=False, _trace=False):
    x = np.asarray(x, np.float32)
    attention_mask = np.asarray(attention_mask)
    if not np.all(attention_mask != 0):
        return _numpy_reference(np.asarray(x), np.asarray(attention_mask),
                                *[np.asarray(a) for a in
                                  (w_q, b_q, w_k, b_k, w_v, b_v, w_o, b_o)])
    w_q, w_k, w_v, w_o = [np.asarray(w, np.float32) for w in (w_q, w_k, w_v, w_o)]
    b_q, b_k, b_v, b_o = [np.asarray(b, np.float32) for b in (b_q, b_k, b_v, b_o)]

    nc = _get_nc(_debug)
    in_maps = _prep_inputs(x, w_q, b_q, w_k, w_v)
    woT_f = np.ascontiguousarray(w_o.T).astype(NPF16)
    for core in range(N_CORES):
        hg = core % 4
        in_maps[core]["woT"] = np.ascontiguousarray(
            woT_f[hg * 256:(hg + 1) * 256, :])

    res = run_bass_kernel_spmd(nc, in_maps, list(range(N_CORES)), trace=_trace)
    const_row = (b_v @ w_o.T + b_o).astype(np.float32)
    y = np.zeros((B, S, D), np.float32)
    for core in range(N_CORES):
        b = core // 4
        y[b] += res.results[core]["y"].astype(np.float32)
    y += const_row
    if _debug or _trace:
        return y, res
    return y


# revision 33
# speedup vs baseline: 1.2248x; 1.1966x over previous
"""Multi-head causal self-attention (B=2, S=2048, D=1024, H=16) on 8 TRN2 cores.

Sharding: core = b*4 + hg  (b in {0,1} batch, hg in {0..3} head-group of 4 heads).
Per core: project qT/kT (pair-packed [128, S], fp16) and v ([S, 64] blocks, fp16),
compute transposed scores S^T = K Q^T per head (k on partitions, two heads per
k-tile via tile_position), exp on ScalarE (both heads in one strided activation,
fp16 out), causal diag masking via one fused strided tensor_mul against a
duplicated upper-tri matrix, PV matmul with a ones-column PREPENDED to V (col 0
of each 128-wide head block, so row 0 of the accumulator is the softmax sum on
partition 0 — the DVE reciprocal mis-executes on HW with a partition-offset
input) and the block zero-padded to 128 columns so the compiler's fast-weight-
load path (NumWeights==128) keeps PV LDWEIGHTS off the critical path.
Normalization per chunk: copy the value rows out of PSUM, reciprocal straight
from the PSUM sums row, then (deferred one-per-kt into the next chunk) a
broadcast of rec and a fused multiply+cast per head. Pair-0 norms broadcast on
the idle GpSimd Q7 ring (PE is ~98% busy there); pair-1 norms broadcast on the
tensor engine (ones[1,64].T @ rec16 — PE has slack there and the 1.8us Q7
launches otherwise sit in pair-1's critical path and re-throttle the PE).
Output projection: per 128-row block both 512-wide halves cast into one
[128,1024] SBUF tile (casts split across DVE and ACT), then a single HWDGE DMA
on the Sync ring per block. Host sums the 4 per-batch partials and adds
(b_v @ w_o.T + b_o); b_k is dropped (softmax is invariant to per-query
constants); b_q is applied on-device.
Startup: inputs stream on the Sync HWDGE ring (cheap RTL descriptor generation;
the gpsimd SWDGE DIRECT2D path costs ~1us of Q7 per trigger and serialized the
old startup) in exact need order with wq split so the first projection chain
starts after ~400KB; the x tail columns + w_o ride the gpsimd ring in dead
time. While the first inputs stream, the PE runs warmup matmuls on a memset
scratch tile so the HAM activity window is full (2.4 GHz) when real work lands.
All matmul operands are fp16 (same PE rate as bf16, 8x the mantissa);
accumulation is fp32 in PSUM.
"""

import numpy as np
import ml_dtypes

import concourse.bass as bass
import concourse.mybir as mybir
import concourse.tile as tile
from concourse import bacc
from concourse.bass_utils import run_bass_kernel_spmd

B, S, D, H, DK = 2, 2048, 1024, 16, 64
N_CORES = 8
F32 = mybir.dt.float32
F16 = mybir.dt.float16
NPF16 = np.float16
AF = mybir.ActivationFunctionType

CHUNKS = [
    [(0, 512), (512, 512), (1024, 512), (1536, 512)],
    [(0, 512), (512, 512), (1024, 512), (1536, 512)],
]

NWARM = 5  # PE warmup matmuls (N=512) during the startup DMA wait


def _build(debug=False):
    nc = bacc.Bacc("TRN2", target_bir_lowering=False, debug=False,
                   num_devices=N_CORES)
    # inputs are host-packed into the exact SBUF tile layouts ("p (c w)"),
    # so every DMA is a partition-contiguous slab (max SDMA efficiency)
    x0p = nc.dram_tensor("x0p", [128, 4096], F16, kind="ExternalInput").ap()
    x1ap = nc.dram_tensor("x1ap", [128, 4096], F16, kind="ExternalInput").ap()
    x1bp = nc.dram_tensor("x1bp", [128, 4096], F16, kind="ExternalInput").ap()
    x1cp = nc.dram_tensor("x1cp", [128, 4096], F16, kind="ExternalInput").ap()
    wqp = nc.dram_tensor("wqp", [128, 2048], F16, kind="ExternalInput").ap()
    wkp = nc.dram_tensor("wkp", [128, 2048], F16, kind="ExternalInput").ap()
    wvp = nc.dram_tensor("wvp", [128, 2048], F16, kind="ExternalInput").ap()
    woT = nc.dram_tensor("woT", [256, D], F16, kind="ExternalInput").ap()
    bq2 = nc.dram_tensor("bq2", [128, 2], F32, kind="ExternalInput").ap()
    tri2 = nc.dram_tensor("tri2", [128, 256], F16, kind="ExternalInput").ap()
    scr = nc.dram_tensor("scr", [2, 1024], F32, kind="Internal").ap()
    y = nc.dram_tensor("y", [S, D], F16, kind="ExternalOutput").ap()
    dbg = {}
    if debug:
        for nm, shp in [("qT", [128, 2, S]), ("kT", [128, 2, S]),
                        ("vv", [128, 16, 260]), ("oT", [128, 2, S])]:
            dbg[nm] = nc.dram_tensor(nm, shp, F16, kind="ExternalOutput").ap()

    NKT = S // 128   # k tiles

    with tile.TileContext(nc) as tc, \
            nc.allow_low_precision(reason="fp16 attention kernel"):
        with (
            tc.tile_pool(name="persist", bufs=1) as persist,
            tc.tile_pool(name="kqv", bufs=2) as kqv,
        ):
            qT_sb = [kqv.tile([128, S], F16, tag="qT", name=f"qT{p}") for p in range(2)]
            kT_sb = [kqv.tile([128, S], F16, tag="kT", name=f"kT{p}") for p in range(2)]
            v_sb = [persist.tile([128, 4 * 65], F16, tag=f"v{t}", name=f"v{t}")
                    for t in range(NKT)]
            outT_sb = [persist.tile([128, S], F16, tag=f"oT{p}", name=f"oTs{p}")
                       for p in range(2)]
            wo_sb = [persist.tile([128, D], F16, tag=f"wo{p}", name=f"wo{p}")
                     for p in range(2)]
            tri_sb = persist.tile([128, 256], F16, tag="tri")
            bq_sb = persist.tile([128, 2], F32, tag="bq")
            ones_sb = persist.tile([1, 64], F16, tag="ones1")
            tri_v = tri_sb.rearrange("p (c w) -> p c w", c=2)

            with (
                tc.tile_pool(name="xw", bufs=1) as xw,
                tc.tile_pool(name="ep", bufs=5) as ep,
                tc.tile_pool(name="rp", bufs=4) as rp,
            ):
                xt0_t = xw.tile([128, 8 * 512], F16, tag="x0", name="xt0")
                xt1a_t = xw.tile([128, 8 * 512], F16, tag="x1a", name="xt1a")
                xt1b_t = xw.tile([128, 8 * 512], F16, tag="x1b", name="xt1b")
                xt1c_t = xw.tile([128, 8 * 512], F16, tag="x1c", name="xt1c")
                wq_t = xw.tile([128, 8 * 256], F16, tag="wq", name="wqs")
                wk_t = xw.tile([128, 8 * 256], F16, tag="wk", name="wks")
                wv_t = xw.tile([128, 8 * 256], F16, tag="wv", name="wvs")
                warm_t = xw.tile([128, 512], F16, tag="warm", name="warm")
                xt0 = xt0_t.rearrange("p (c w) -> p c w", c=8)
                xt1v = [t.rearrange("p (c w) -> p c w", c=8)
                        for t in (xt1a_t, xt1b_t, xt1c_t)]
                wq_v = wq_t.rearrange("p (c w) -> p c w", c=8)
                wk_v = wk_t.rearrange("p (c w) -> p c w", c=8)
                wv_v = wv_t.rearrange("p (c w) -> p c w", c=8)

                # Two HWDGE rings in parallel (each sustains only ~half the
                # HBM wire), both in need order: x-side on sync, weight-side
                # on scalar; tri alone on the gpsimd SWDGE ring. Every
                # transfer is a contiguous slab of a host-packed tensor.
                nc.sync.dma_start(out=xt0_t[:, 0:1024], in_=x0p[:, 0:1024])
                nc.sync.dma_start(out=xt0_t[:, 1024:2048], in_=x0p[:, 1024:2048])
                nc.sync.dma_start(out=xt0_t[:, 2048:4096], in_=x0p[:, 2048:4096])
                nc.sync.dma_start(out=xt1a_t, in_=x1ap)
                nc.sync.dma_start(out=xt1b_t, in_=x1bp)
                nc.sync.dma_start(out=xt1c_t, in_=x1cp)
                nc.scalar.dma_start(out=bq_sb, in_=bq2)
                nc.scalar.dma_start(out=wq_t[:, 0:512], in_=wqp[:, 0:512])
                nc.scalar.dma_start(out=wq_t[:, 512:2048], in_=wqp[:, 512:2048])
                nc.scalar.dma_start(out=wk_t, in_=wkp)
                nc.scalar.dma_start(out=wv_t, in_=wvp)
                for p in range(2):
                    nc.scalar.dma_start(out=wo_sb[p], in_=woT[p * 128:(p + 1) * 128, :])
                nc.gpsimd.dma_start(out=tri_sb, in_=tri2)

                nc.vector.memset(ones_sb, 1.0)
                nc.vector.memset(warm_t, 0.125)
                # preload the exp table set (~2.7us ACT_TABLE_LOAD) during the
                # startup DMA wait instead of before the first real exp
                dummy_e = ep.tile([1, 64], F16, tag="e", name="dummy_e")
                nc.scalar.activation(dummy_e, warm_t[0:1, 0:64], AF.Exp,
                                     scale=0.125)

                def xcols(c, a, b):
                    blk = a // 512
                    assert b <= (blk + 1) * 512
                    if blk == 0:
                        return xt0[:, c, a:b]
                    t = xt1v[blk - 1]
                    return t[:, c, a - blk * 512:b - blk * 512]

                def qk_chain(p, j, which, pool, tag="proj"):
                    ps = pool.tile([128, 512], F32, tag=tag, name="ps")
                    w_v_ = wq_v if which == "q" else wk_v
                    for c in range(8):
                        nc.tensor.matmul(
                            ps, w_v_[:, c, p * 128:(p + 1) * 128],
                            xcols(c, j * 512, (j + 1) * 512),
                            start=(c == 0), stop=(c == 7))
                    if which == "q":
                        nc.vector.tensor_scalar_add(
                            qT_sb[p][:, j * 512:(j + 1) * 512], ps,
                            bq_sb[:, p:p + 1])
                    else:
                        nc.vector.tensor_copy(
                            kT_sb[p][:, j * 512:(j + 1) * 512], ps)

                def v_chain(t, pool):
                    ps_v = pool.tile([128, 256], F32, tag="proj", name="ps_v")
                    for c in range(8):
                        nc.tensor.matmul(
                            ps_v, xcols(c, t * 128, (t + 1) * 128), wv_v[:, c, :],
                            start=(c == 0), stop=(c == 7))
                    v_view = v_sb[t].rearrange("p (h w) -> p h w", w=65)
                    nc.vector.memset(v_view[:, :, 64:65], 1.0)
                    nc.vector.tensor_copy(
                        v_view[:, :, 0:64],
                        ps_v.rearrange("p (h w) -> p h w", w=64))

                norm_rest = []

                def emit_norm(p_, q0_, a_, b_, o_ps_, defer=True,
                              mm_bcast=False):
                    # copy values + the sums row out of PSUM for chunk columns
                    # [a, b); sums staged to partition 0 (PSUM reads must
                    # start partition-aligned, and the DVE reciprocal
                    # mis-executes on HW with a partition-offset input).
                    # The broadcast launches HERE (Q7 is idle; its ~1.8us
                    # latency hides across the chunk boundary) — only the
                    # final multiplies defer into the next chunk's slots. The
                    # staged last-chunk norms broadcast on the tensor engine
                    # instead (zero added latency before the output tail).
                    w_ = b_ - a_
                    rec = rp.tile([1, 1024], F32, tag="rec", name="recip")
                    sums = rp.tile([1, 1024], F32, tag="sums", name="sums")
                    ocs = []
                    for s in range(2):
                        oc = rp.tile([64, 512], F32, tag=f"oc{s}", name="o_cp")
                        nc.vector.tensor_copy(oc[:, 0:w_], o_ps_[s][0:64, a_:b_])
                        nc.vector.tensor_copy(
                            sums[:, s * 512:s * 512 + w_],
                            o_ps_[s][64:65, a_:b_])
                        ocs.append(oc)
                    for s in range(2):
                        nc.vector.reciprocal_approx_fast(
                            out=rec[:, s * 512:s * 512 + w_],
                            in_=sums[:, s * 512:s * 512 + w_])
                    if mm_bcast:
                        rec16 = rp.tile([1, 1024], F16, tag="rec16", name="r16")
                        nc.vector.tensor_copy(rec16, rec)
                        bc_list = []
                        for s in range(2):
                            bc_ps = fps.tile([128, 512], F32, tag="f", name="bcp")
                            nc.tensor.matmul(
                                bc_ps[0:64, :], ones_sb,
                                rec16[:, s * 512:(s + 1) * 512],
                                start=True, stop=True, skip_group_check=True)
                            bc_list.append(bc_ps[0:64, :])
                    else:
                        bcs = rp.tile([64, 1024], F32, tag="bc", name="bc")
                        nc.gpsimd.partition_broadcast(bcs, rec)
                        bc_list = [bcs[:, s * 512:(s + 1) * 512] for s in range(2)]
                    if defer:
                        norm_rest.append(
                            lambda: finish_norm(p_, q0_ + a_, w_, bc_list, ocs))
                    else:
                        finish_norm(p_, q0_ + a_, w_, bc_list, ocs)

                # NOTE: the multiplies must stay on the DVE — running them on
                # GpSimd interleaves Multiply with PartitionBroadcast and the
                # Q7 LIBRARY_RELOAD churn between op types costs multiple us
                # of hidden latency per switch (measured: 17.9us stalls).
                def finish_norm(p_, qa_, w_, bc_list, ocs):
                    for s in range(2):
                        nc.vector.tensor_mul(
                            outT_sb[p_][s * 64:(s + 1) * 64, qa_:qa_ + w_],
                            ocs[s][:, 0:w_],
                            bc_list[s][:, 0:w_])

                def emit_pair(p, fillers, stage=None):
                    # stage: (split_col, hookA, hookB) applied to the LAST
                    # chunk: columns [0, split) normalize right after their
                    # final PV (two k-tiles early), so most of the output
                    # projection tail overlaps the chunk's trailing k-tiles.
                    nchunks = len(CHUNKS[p])
                    for ci, (q0, qcw) in enumerate(CHUNKS[p]):
                        nkt = (q0 + qcw) // 128
                        last = stage is not None and ci == nchunks - 1
                        o_ps = [opp.tile([65, 512], F32, tag=f"o{s}", name=f"ops{s}")
                                for s in range(2)]
                        pend = None
                        for kt in range(nkt):
                            o = kt * 128 - q0
                            diag = o >= 0
                            lo = o if diag else 0
                            s_ab = sqp.tile([128, 1024], F32, tag="sq", name="s_ab")
                            s_v = s_ab.rearrange("p (c w) -> p c w", c=2)
                            for s in range(2):
                                nc.tensor.matmul(
                                    s_v[:, s, lo:qcw],
                                    kT_sb[p][s * 64:(s + 1) * 64,
                                             kt * 128:(kt + 1) * 128],
                                    qT_sb[p][s * 64:(s + 1) * 64,
                                             q0 + lo:q0 + qcw],
                                    start=True, stop=True,
                                    tile_position=(s * 64, 0),
                                    skip_group_check=True)
                            e_ab = ep.tile([128, 1024], F16, tag="e", name="e_ab")
                            e_v = e_ab.rearrange("p (c w) -> p c w", c=2)
                            nc.scalar.activation(
                                e_v[:, :, lo:qcw], s_v[:, :, lo:qcw],
                                AF.Exp, scale=0.125)
                            if diag:
                                nc.vector.tensor_mul(
                                    e_v[:, :, o:o + 128], e_v[:, :, o:o + 128],
                                    tri_v)
                            if norm_rest:
                                norm_rest.pop(0)()
                            if fillers is not None:
                                fillers(ci, kt, nkt)
                            if pend is not None:
                                _kt, _e, _lo = pend
                                for s in range(2):
                                    hb = 2 * p + s
                                    nc.tensor.matmul(
                                        o_ps[s][:, _lo:qcw],
                                        v_sb[_kt][:, hb * 65:(hb + 1) * 65],
                                        _e[:, s, _lo:qcw],
                                        start=(_kt == 0), stop=False,
                                        skip_group_check=True)
                            pend = (kt, e_v, lo)
                            if last and kt == nkt - 2:
                                # PV(kt-1) just issued; chunk columns
                                # [0, split) are final in PSUM
                                split, hookA, _ = stage
                                emit_norm(p, q0, 0, split, o_ps, defer=False,
                                          mm_bcast=True)
                                hookA()
                        _kt, _e, _lo = pend
                        for s in range(2):
                            hb = 2 * p + s
                            nc.tensor.matmul(
                                o_ps[s][:, _lo:qcw],
                                v_sb[_kt][:, hb * 65:(hb + 1) * 65],
                                _e[:, s, _lo:qcw],
                                start=False, stop=True,
                                skip_group_check=True)
                        if last:
                            split, _, hookB = stage
                            emit_norm(p, q0, split, qcw, o_ps, defer=False,
                                      mm_bcast=True)
                            hookB()
                        else:
                            emit_norm(p, q0, 0, qcw, o_ps)
                        yield ci
                    while norm_rest:
                        norm_rest.pop(0)()

                # ---- PE warmup + pair 0 front: j0 q/k chains ----
                with tc.tile_pool(name="ppsA", bufs=2, space="PSUM") as ppsA:
                    ps_q0 = ppsA.tile([128, 512], F32, tag="projA", name="ps_q0")
                    ps_k0 = ppsA.tile([128, 512], F32, tag="projA", name="ps_k0")
                    for i in range(NWARM):
                        nc.tensor.matmul(ps_q0, warm_t[:, 0:128], warm_t,
                                         start=True, stop=True,
                                         skip_group_check=True)
                    for c in range(8):
                        nc.tensor.matmul(
                            ps_q0, wq_v[:, c, 0:128], xt0[:, c, :],
                            start=(c == 0), stop=(c == 7))
                    nc.vector.tensor_scalar_add(
                        qT_sb[0][:, 0:512], ps_q0, bq_sb[:, 0:1])
                    # warmup bridges: dependency-free matmuls that keep the PE
                    # busy (HAM stays at full clock) across the DMA-paced
                    # waits for wk / the q,kT copies before chunk-0 scores
                    for i in range(3):
                        nc.tensor.matmul(ps_q0, warm_t[:, 0:128], warm_t,
                                         start=True, stop=True,
                                         skip_group_check=True)
                    for c in range(8):
                        nc.tensor.matmul(
                            ps_k0, wk_v[:, c, 0:128], xt0[:, c, :],
                            start=(c == 0), stop=(c == 7))
                    nc.vector.tensor_copy(kT_sb[0][:, 0:512], ps_k0)
                    for i in range(2):
                        nc.tensor.matmul(ps_k0, warm_t[:, 0:128], warm_t,
                                         start=True, stop=True,
                                         skip_group_check=True)
                with (
                    tc.tile_pool(name="sq", bufs=2, space="PSUM") as sqp,
                    tc.tile_pool(name="ops", bufs=1, space="PSUM") as opp,
                ):
                    with tc.tile_pool(name="pps", bufs=2, space="PSUM") as pps:
                        # pair-0 fillers, placed to match DMA arrivals:
                        # chunk 0: v0-1 + pair-0 j1 (x block 1 lands mid-chunk)
                        # chunk 1: v2-7 + pair-0 j2
                        # chunk 2: v8-11 + pair-0 j3
                        # chunk 3: v12-15 + pair-1 j0 (pair-1 j1-j3 moved into
                        #          pair-1's own slots — pair 0 runs PE-
                        #          saturated while pair 1 has idle slots)
                        def qkf(p_, j_):
                            return [lambda: qk_chain(p_, j_, "q", pps),
                                    lambda: qk_chain(p_, j_, "k", pps)]

                        def vf(a_, b_):
                            return [lambda t=t: v_chain(t, pps)
                                    for t in range(a_, b_)]

                        # v(t) must be emitted in or before the chunk whose PV
                        # consumes it (chunk ci's PV covers k-tiles < 4*(ci+1))
                        sched = {0: vf(0, 4) + qkf(0, 1),
                                 1: vf(4, 8) + qkf(0, 2),
                                 2: vf(8, 12) + qkf(0, 3),
                                 3: vf(12, 16) + qkf(1, 0)}
                        queues = [list(sched.get(ci, [])) for ci in range(4)]

                        def filler_pop(ci, kt, nkt):
                            q = queues[ci]
                            rem_slots = nkt - kt
                            while q and len(q) >= rem_slots:
                                q.pop(0)()
                            if q:
                                q.pop(0)()

                        for _ci in emit_pair(0, filler_pop):
                            while queues[_ci]:
                                queues[_ci].pop(0)()

                    if debug:
                        for p in range(2):
                            nc.sync.dma_start(out=dbg["qT"][:, p, :], in_=qT_sb[p])
                            nc.sync.dma_start(out=dbg["kT"][:, p, :], in_=kT_sb[p])
                        for t in range(NKT):
                            nc.sync.dma_start(out=dbg["vv"][:, t, :], in_=v_sb[t])

                    # ---- pair 1: output projection as fillers ----
                    with tc.tile_pool(name="fps", bufs=2, space="PSUM") as fps:
                        with tc.tile_pool(name="fsb", bufs=4) as fsb:
                            c_alt = [0]
                            fsb_tiles = {}

                            def c_unit(qt, oc_i, vec=None):
                                f_ps = fps.tile([128, 512], F32, tag="f", name="f_ps")
                                for p in range(2):
                                    nc.tensor.matmul(
                                        f_ps, outT_sb[p][:, qt * 128:(qt + 1) * 128],
                                        wo_sb[p][:, oc_i * 512:(oc_i + 1) * 512],
                                        start=(p == 0), stop=(p == 1))
                                if qt not in fsb_tiles:
                                    fsb_tiles[qt] = fsb.tile(
                                        [128, 1024], F16, tag="f", name=f"fsb{qt}")
                                half = fsb_tiles[qt][:, oc_i * 512:(oc_i + 1) * 512]
                                c_alt[0] ^= 1
                                use_vec = vec if vec is not None else c_alt[0]
                                if use_vec:
                                    nc.vector.tensor_copy(half, f_ps)
                                else:
                                    nc.scalar.activation(half, f_ps, AF.Identity)
                                if oc_i == 1:
                                    # one HWDGE DMA per 128-row output block,
                                    # alternating the two HWDGE rings
                                    eng = nc.sync if qt % 2 == 0 else nc.scalar
                                    eng.dma_start(
                                        out=y[qt * 128:(qt + 1) * 128, :],
                                        in_=fsb_tiles.pop(qt))

                            NCH = len(CHUNKS[1])
                            # pair-1's own j1-j3 projection chains run as
                            # early-chunk fillers here (chunk ci's scores only
                            # need q/kT cols up to (ci+1)*512, so j(ci+1)
                            # finishing inside chunk ci is in time)
                            cqueues = [[] for _ in range(NCH)]
                            for ci in range(3):
                                if ci + 1 <= 3:
                                    cqueues[ci] += [
                                        lambda j=ci + 1: qk_chain(1, j, "q", fps, tag="f"),
                                        lambda j=ci + 1: qk_chain(1, j, "k", fps, tag="f")]

                            def c_pop(ci, kt, nkt):
                                # front-load: chunk boundaries are where pair-1
                                # PE gaps cluster, so pop extra fillers early
                                q = cqueues[ci]
                                rem_slots = nkt - kt
                                while q and len(q) >= rem_slots:
                                    q.pop(0)()
                                for _ in range(2 if kt < 3 else 1):
                                    if q:
                                        q.pop(0)()

                            qt_ranges = [(0, 4), (4, 8), (8, 12)]

                            def tail_hook_a():
                                # qt 12-13 project while k-tiles 14/15 finish;
                                # all casts on DVE — a scalar-engine cast here
                                # would queue behind the last exps (FIFO)
                                for qt in (12, 13):
                                    c_unit(qt, 0, vec=True)
                                    c_unit(qt, 1, vec=True)

                            def tail_hook_b():
                                for qt in (14, 15):
                                    c_unit(qt, 0, vec=True)
                                    c_unit(qt, 1, vec=True)

                            for ci in emit_pair(1, c_pop,
                                                stage=(256, tail_hook_a,
                                                       tail_hook_b)):
                                if ci < len(qt_ranges):
                                    a, b = qt_ranges[ci]
                                    units = []
                                    for qt in range(a, b):
                                        for oc_i in range(2):
                                            units.append(
                                                lambda qt=qt, oc_i=oc_i:
                                                c_unit(qt, oc_i))
                                    cqueues[ci + 1].extend(units)
                            for q in cqueues:
                                while q:
                                    q.pop(0)()

            if debug:
                for p in range(2):
                    nc.sync.dma_start(out=dbg["oT"][:, p, :], in_=outT_sb[p])

    nc.compile()
    return nc


_cached = {}


def _get_nc(debug=False):
    key = bool(debug)
    if key not in _cached:
        _cached[key] = _build(debug)
    return _cached[key]


def _pack_pcw(a):
    # [(c p), w] -> [p, (c w)]  (the SBUF tile layout)
    cp, w = a.shape
    c = cp // 128
    return np.ascontiguousarray(
        a.reshape(c, 128, w).transpose(1, 0, 2).reshape(128, c * w))


def _prep_inputs(x, w_q, b_q, w_k, w_v):
    tri = np.triu(np.ones((128, 128), np.float32)).astype(NPF16)
    tri2 = np.concatenate([tri, tri], axis=1)
    wqT_f = np.ascontiguousarray(w_q.T).astype(NPF16)
    wkT_f = np.ascontiguousarray(w_k.T).astype(NPF16)
    wvT_f = np.ascontiguousarray(w_v.T).astype(NPF16)
    in_maps = []
    for core in range(N_CORES):
        b, hg = divmod(core, 4)
        cs = slice(hg * 256, (hg + 1) * 256)
        xm = x[b].T.astype(NPF16)  # [D, S]
        in_maps.append({
            "x0p": _pack_pcw(xm[:, 0:512]),
            "x1ap": _pack_pcw(xm[:, 512:1024]),
            "x1bp": _pack_pcw(xm[:, 1024:1536]),
            "x1cp": _pack_pcw(xm[:, 1536:2048]),
            "wqp": _pack_pcw(wqT_f[:, cs]),
            "wkp": _pack_pcw(wkT_f[:, cs]),
            "wvp": _pack_pcw(wvT_f[:, cs]),
            "bq2": np.ascontiguousarray(
                b_q[hg * 256:(hg + 1) * 256].reshape(2, 128).T.astype(np.float32)),
            "tri2": tri2,
        })
    return in_maps


def _numpy_reference(x, attention_mask, w_q, b_q, w_k, b_k, w_v, b_v, w_o, b_o):
    x = x.astype(np.float64)
    q = (x @ w_q.T + b_q).reshape(B, S, H, DK).transpose(0, 2, 1, 3)
    k = (x @ w_k.T + b_k).reshape(B, S, H, DK).transpose(0, 2, 1, 3)
    v = (x @ w_v.T + b_v).reshape(B, S, H, DK).transpose(0, 2, 1, 3)
    scores = np.einsum("bhqd,bhkd->bhqk", q, k, optimize=True) / np.sqrt(DK)
    causal = np.tril(np.ones((S, S), bool))
    mask = causal[None, None] & (attention_mask[:, None, None, :] != 0)
    scores = np.where(mask, scores, -np.inf)
    scores -= scores.max(-1, keepdims=True)
    e = np.exp(scores)
    attn = e / e.sum(-1, keepdims=True)
    out = np.einsum("bhqk,bhkd->bhqd", attn, v, optimize=True)
    out = out.transpose(0, 2, 1, 3).reshape(B, S, D)
    return (out @ w_o.T + b_o).astype(np.float32)


def kernel(x, attention_mask, w_q, b_q, w_k, b_k, w_v, b_v, w_o, b_o,
           _debug=False, _trace=False):
    x = np.asarray(x, np.float32)
    attention_mask = np.asarray(attention_mask)
    if not np.all(attention_mask != 0):
        return _numpy_reference(np.asarray(x), np.asarray(attention_mask),
                                *[np.asarray(a) for a in
                                  (w_q, b_q, w_k, b_k, w_v, b_v, w_o, b_o)])
    w_q, w_k, w_v, w_o = [np.asarray(w, np.float32) for w in (w_q, w_k, w_v, w_o)]
    b_q, b_k, b_v, b_o = [np.asarray(b, np.float32) for b in (b_q, b_k, b_v, b_o)]

    nc = _get_nc(_debug)
    in_maps = _prep_inputs(x, w_q, b_q, w_k, w_v)
    woT_f = np.ascontiguousarray(w_o.T).astype(NPF16)
    for core in range(N_CORES):
        hg = core % 4
        in_maps[core]["woT"] = np.ascontiguousarray(
            woT_f[hg * 256:(hg + 1) * 256, :])

    res = run_bass_kernel_spmd(nc, in_maps, list(range(N_CORES)), trace=_trace)
    const_row = (b_v @ w_o.T + b_o).astype(np.float32)
    y = np.zeros((B, S, D), np.float32)
    for core in range(N_CORES):
        b = core // 4
        y[b] += res.results[core]["y"].astype(np.float32)
    y += const_row
    if _debug or _trace:
        return y, res
    return y


# revision 35
# speedup vs baseline: 1.2261x; 1.0010x over previous
"""Multi-head causal self-attention (B=2, S=2048, D=1024, H=16) on 8 TRN2 cores.

Sharding: core = b*4 + hg  (b in {0,1} batch, hg in {0..3} head-group of 4 heads).
Per core: project qT/kT (pair-packed [128, S], fp16) and v ([S, 64] blocks, fp16),
compute transposed scores S^T = K Q^T per head (k on partitions, two heads per
k-tile via tile_position), exp on ScalarE (both heads in one strided activation,
fp16 out), causal diag masking via one fused strided tensor_mul against a
duplicated upper-tri matrix, PV matmul with a ones-column PREPENDED to V (col 0
of each 128-wide head block, so row 0 of the accumulator is the softmax sum on
partition 0 — the DVE reciprocal mis-executes on HW with a partition-offset
input) and the block zero-padded to 128 columns so the compiler's fast-weight-
load path (NumWeights==128) keeps PV LDWEIGHTS off the critical path.
Normalization per chunk: copy the value rows out of PSUM, reciprocal straight
from the PSUM sums row, then (deferred one-per-kt into the next chunk) a
broadcast of rec and a fused multiply+cast per head. Pair-0 norms broadcast on
the idle GpSimd Q7 ring (PE is ~98% busy there); pair-1 norms broadcast on the
tensor engine (ones[1,64].T @ rec16 — PE has slack there and the 1.8us Q7
launches otherwise sit in pair-1's critical path and re-throttle the PE).
Output projection: per 128-row block both 512-wide halves cast into one
[128,1024] SBUF tile (casts split across DVE and ACT), then a single HWDGE DMA
on the Sync ring per block. Host sums the 4 per-batch partials and adds
(b_v @ w_o.T + b_o); b_k is dropped (softmax is invariant to per-query
constants); b_q is applied on-device.
Startup: inputs stream on the Sync HWDGE ring (cheap RTL descriptor generation;
the gpsimd SWDGE DIRECT2D path costs ~1us of Q7 per trigger and serialized the
old startup) in exact need order with wq split so the first projection chain
starts after ~400KB; the x tail columns + w_o ride the gpsimd ring in dead
time. While the first inputs stream, the PE runs warmup matmuls on a memset
scratch tile so the HAM activity window is full (2.4 GHz) when real work lands.
All matmul operands are fp16 (same PE rate as bf16, 8x the mantissa);
accumulation is fp32 in PSUM.
"""

import numpy as np
import ml_dtypes

import concourse.bass as bass
import concourse.mybir as mybir
import concourse.tile as tile
from concourse import bacc
from concourse.bass_utils import run_bass_kernel_spmd

B, S, D, H, DK = 2, 2048, 1024, 16, 64
N_CORES = 8
F32 = mybir.dt.float32
F16 = mybir.dt.float16
NPF16 = np.float16
AF = mybir.ActivationFunctionType

CHUNKS = [
    [(0, 512), (512, 512), (1024, 512), (1536, 512)],
    [(0, 512), (512, 512), (1024, 512), (1536, 512)],
]

NWARM = 5  # PE warmup matmuls (N=512) during the startup DMA wait


def _build(debug=False):
    nc = bacc.Bacc("TRN2", target_bir_lowering=False, debug=False,
                   num_devices=N_CORES)
    # inputs are host-packed into the exact SBUF tile layouts ("p (c w)"),
    # so every DMA is a partition-contiguous slab (max SDMA efficiency)
    x0p = nc.dram_tensor("x0p", [128, 4096], F16, kind="ExternalInput").ap()
    x1ap = nc.dram_tensor("x1ap", [128, 4096], F16, kind="ExternalInput").ap()
    x1bp = nc.dram_tensor("x1bp", [128, 4096], F16, kind="ExternalInput").ap()
    x1cp = nc.dram_tensor("x1cp", [128, 4096], F16, kind="ExternalInput").ap()
    wqp = nc.dram_tensor("wqp", [128, 2048], F16, kind="ExternalInput").ap()
    wkp = nc.dram_tensor("wkp", [128, 2048], F16, kind="ExternalInput").ap()
    wvp = nc.dram_tensor("wvp", [128, 2048], F16, kind="ExternalInput").ap()
    woT = nc.dram_tensor("woT", [256, D], F16, kind="ExternalInput").ap()
    bq2 = nc.dram_tensor("bq2", [128, 2], F32, kind="ExternalInput").ap()
    tri2 = nc.dram_tensor("tri2", [128, 256], F16, kind="ExternalInput").ap()
    scr = nc.dram_tensor("scr", [2, 1024], F32, kind="Internal").ap()
    y = nc.dram_tensor("y", [S, D], F16, kind="ExternalOutput").ap()
    dbg = {}
    if debug:
        for nm, shp in [("qT", [128, 2, S]), ("kT", [128, 2, S]),
                        ("vv", [128, 16, 260]), ("oT", [128, 2, S])]:
            dbg[nm] = nc.dram_tensor(nm, shp, F16, kind="ExternalOutput").ap()

    NKT = S // 128   # k tiles

    with tile.TileContext(nc) as tc, \
            nc.allow_low_precision(reason="fp16 attention kernel"):
        with (
            tc.tile_pool(name="persist", bufs=1) as persist,
            tc.tile_pool(name="kqv", bufs=2) as kqv,
        ):
            qT_sb = [kqv.tile([128, S], F16, tag="qT", name=f"qT{p}") for p in range(2)]
            kT_sb = [kqv.tile([128, S], F16, tag="kT", name=f"kT{p}") for p in range(2)]
            v_sb = [persist.tile([128, 4 * 65], F16, tag=f"v{t}", name=f"v{t}")
                    for t in range(NKT)]
            outT_sb = [persist.tile([128, S], F16, tag=f"oT{p}", name=f"oTs{p}")
                       for p in range(2)]
            wo_sb = [persist.tile([128, D], F16, tag=f"wo{p}", name=f"wo{p}")
                     for p in range(2)]
            tri_sb = persist.tile([128, 256], F16, tag="tri")
            bq_sb = persist.tile([128, 2], F32, tag="bq")
            ones_sb = persist.tile([1, 64], F16, tag="ones1")
            tri_v = tri_sb.rearrange("p (c w) -> p c w", c=2)

            with (
                tc.tile_pool(name="xw", bufs=1) as xw,
                tc.tile_pool(name="ep", bufs=5) as ep,
                tc.tile_pool(name="rp", bufs=4) as rp,
            ):
                xt0_t = xw.tile([128, 8 * 512], F16, tag="x0", name="xt0")
                xt1a_t = xw.tile([128, 8 * 512], F16, tag="x1a", name="xt1a")
                xt1b_t = xw.tile([128, 8 * 512], F16, tag="x1b", name="xt1b")
                xt1c_t = xw.tile([128, 8 * 512], F16, tag="x1c", name="xt1c")
                wq_t = xw.tile([128, 8 * 256], F16, tag="wq", name="wqs")
                wk_t = xw.tile([128, 8 * 256], F16, tag="wk", name="wks")
                wv_t = xw.tile([128, 8 * 256], F16, tag="wv", name="wvs")
                warm_t = xw.tile([128, 512], F16, tag="warm", name="warm")
                xt0 = xt0_t.rearrange("p (c w) -> p c w", c=8)
                xt1v = [t.rearrange("p (c w) -> p c w", c=8)
                        for t in (xt1a_t, xt1b_t, xt1c_t)]
                wq_v = wq_t.rearrange("p (c w) -> p c w", c=8)
                wk_v = wk_t.rearrange("p (c w) -> p c w", c=8)
                wv_v = wv_t.rearrange("p (c w) -> p c w", c=8)

                # Two HWDGE rings in parallel (each sustains only ~half the
                # HBM wire), both in need order: x-side on sync, weight-side
                # on scalar; tri alone on the gpsimd SWDGE ring. Every
                # transfer is a contiguous slab of a host-packed tensor.
                nc.sync.dma_start(out=xt0_t[:, 0:1024], in_=x0p[:, 0:1024])
                nc.sync.dma_start(out=xt0_t[:, 1024:2048], in_=x0p[:, 1024:2048])
                nc.sync.dma_start(out=xt0_t[:, 2048:4096], in_=x0p[:, 2048:4096])
                nc.sync.dma_start(out=xt1a_t, in_=x1ap)
                nc.sync.dma_start(out=xt1b_t, in_=x1bp)
                nc.sync.dma_start(out=xt1c_t, in_=x1cp)
                nc.scalar.dma_start(out=bq_sb, in_=bq2)
                nc.scalar.dma_start(out=wq_t[:, 0:512], in_=wqp[:, 0:512])
                nc.scalar.dma_start(out=wq_t[:, 512:2048], in_=wqp[:, 512:2048])
                nc.scalar.dma_start(out=wk_t, in_=wkp)
                nc.scalar.dma_start(out=wv_t, in_=wvp)
                for p in range(2):
                    nc.scalar.dma_start(out=wo_sb[p], in_=woT[p * 128:(p + 1) * 128, :])
                nc.gpsimd.dma_start(out=tri_sb, in_=tri2)

                nc.vector.memset(ones_sb, 1.0)
                nc.vector.memset(warm_t, 0.125)
                # preload the exp table set (~2.7us ACT_TABLE_LOAD) during the
                # startup DMA wait instead of before the first real exp
                dummy_e = ep.tile([1, 64], F16, tag="e", name="dummy_e")
                nc.scalar.activation(dummy_e, warm_t[0:1, 0:64], AF.Exp,
                                     scale=0.125)

                def xcols(c, a, b):
                    blk = a // 512
                    assert b <= (blk + 1) * 512
                    if blk == 0:
                        return xt0[:, c, a:b]
                    t = xt1v[blk - 1]
                    return t[:, c, a - blk * 512:b - blk * 512]

                def qk_chain(p, j, which, pool, tag="proj"):
                    ps = pool.tile([128, 512], F32, tag=tag, name="ps")
                    w_v_ = wq_v if which == "q" else wk_v
                    for c in range(8):
                        nc.tensor.matmul(
                            ps, w_v_[:, c, p * 128:(p + 1) * 128],
                            xcols(c, j * 512, (j + 1) * 512),
                            start=(c == 0), stop=(c == 7))
                    if which == "q":
                        nc.vector.tensor_scalar_add(
                            qT_sb[p][:, j * 512:(j + 1) * 512], ps,
                            bq_sb[:, p:p + 1])
                    else:
                        nc.vector.tensor_copy(
                            kT_sb[p][:, j * 512:(j + 1) * 512], ps)

                def v_chain(t, pool):
                    ps_v = pool.tile([128, 256], F32, tag="proj", name="ps_v")
                    for c in range(8):
                        nc.tensor.matmul(
                            ps_v, xcols(c, t * 128, (t + 1) * 128), wv_v[:, c, :],
                            start=(c == 0), stop=(c == 7))
                    v_view = v_sb[t].rearrange("p (h w) -> p h w", w=65)
                    nc.vector.memset(v_view[:, :, 64:65], 1.0)
                    nc.vector.tensor_copy(
                        v_view[:, :, 0:64],
                        ps_v.rearrange("p (h w) -> p h w", w=64))

                norm_rest = []

                def emit_norm(p_, q0_, a_, b_, o_ps_, defer=True,
                              mm_bcast=False):
                    # copy values + the sums row out of PSUM for chunk columns
                    # [a, b); sums staged to partition 0 (PSUM reads must
                    # start partition-aligned, and the DVE reciprocal
                    # mis-executes on HW with a partition-offset input).
                    # The broadcast launches HERE (Q7 is idle; its ~1.8us
                    # latency hides across the chunk boundary) — only the
                    # final multiplies defer into the next chunk's slots. The
                    # staged last-chunk norms broadcast on the tensor engine
                    # instead (zero added latency before the output tail).
                    w_ = b_ - a_
                    rec = rp.tile([1, 1024], F32, tag="rec", name="recip")
                    sums = rp.tile([1, 1024], F32, tag="sums", name="sums")
                    ocs = []
                    for s in range(2):
                        oc = rp.tile([64, 512], F32, tag=f"oc{s}", name="o_cp")
                        nc.vector.tensor_copy(oc[:, 0:w_], o_ps_[s][0:64, a_:b_])
                        nc.vector.tensor_copy(
                            sums[:, s * 512:s * 512 + w_],
                            o_ps_[s][64:65, a_:b_])
                        ocs.append(oc)
                    for s in range(2):
                        nc.vector.reciprocal_approx_fast(
                            out=rec[:, s * 512:s * 512 + w_],
                            in_=sums[:, s * 512:s * 512 + w_])
                    if mm_bcast:
                        rec16 = rp.tile([1, 1024], F16, tag="rec16", name="r16")
                        nc.vector.tensor_copy(rec16, rec)
                        bc_list = []
                        for s in range(2):
                            bc_ps = fps.tile([128, 512], F32, tag="f", name="bcp")
                            nc.tensor.matmul(
                                bc_ps[0:64, :], ones_sb,
                                rec16[:, s * 512:(s + 1) * 512],
                                start=True, stop=True, skip_group_check=True)
                            bc_list.append(bc_ps[0:64, :])
                    else:
                        bcs = rp.tile([64, 1024], F32, tag="bc", name="bc")
                        nc.gpsimd.partition_broadcast(bcs, rec)
                        bc_list = [bcs[:, s * 512:(s + 1) * 512] for s in range(2)]
                    if defer:
                        norm_rest.append(
                            lambda: finish_norm(p_, q0_ + a_, w_, bc_list, ocs))
                    else:
                        finish_norm(p_, q0_ + a_, w_, bc_list, ocs)

                # NOTE: the multiplies must stay on the DVE — running them on
                # GpSimd interleaves Multiply with PartitionBroadcast and the
                # Q7 LIBRARY_RELOAD churn between op types costs multiple us
                # of hidden latency per switch (measured: 17.9us stalls).
                def finish_norm(p_, qa_, w_, bc_list, ocs):
                    for s in range(2):
                        nc.vector.tensor_mul(
                            outT_sb[p_][s * 64:(s + 1) * 64, qa_:qa_ + w_],
                            ocs[s][:, 0:w_],
                            bc_list[s][:, 0:w_])

                def emit_pair(p, fillers, stage=None):
                    # stage: (split_col, hookA, hookB) applied to the LAST
                    # chunk: columns [0, split) normalize right after their
                    # final PV (two k-tiles early), so most of the output
                    # projection tail overlaps the chunk's trailing k-tiles.
                    nchunks = len(CHUNKS[p])
                    for ci, (q0, qcw) in enumerate(CHUNKS[p]):
                        nkt = (q0 + qcw) // 128
                        last = stage is not None and ci == nchunks - 1
                        o_ps = [opp.tile([65, 512], F32, tag=f"o{s}", name=f"ops{s}")
                                for s in range(2)]
                        pend = None
                        for kt in range(nkt):
                            o = kt * 128 - q0
                            diag = o >= 0
                            lo = o if diag else 0
                            s_ab = sqp.tile([128, 1024], F32, tag="sq", name="s_ab")
                            s_v = s_ab.rearrange("p (c w) -> p c w", c=2)
                            for s in range(2):
                                nc.tensor.matmul(
                                    s_v[:, s, lo:qcw],
                                    kT_sb[p][s * 64:(s + 1) * 64,
                                             kt * 128:(kt + 1) * 128],
                                    qT_sb[p][s * 64:(s + 1) * 64,
                                             q0 + lo:q0 + qcw],
                                    start=True, stop=True,
                                    tile_position=(s * 64, 0),
                                    skip_group_check=True)
                            e_ab = ep.tile([128, 1024], F16, tag="e", name="e_ab")
                            e_v = e_ab.rearrange("p (c w) -> p c w", c=2)
                            nc.scalar.activation(
                                e_v[:, :, lo:qcw], s_v[:, :, lo:qcw],
                                AF.Exp, scale=0.125)
                            if diag:
                                nc.vector.tensor_mul(
                                    e_v[:, :, o:o + 128], e_v[:, :, o:o + 128],
                                    tri_v)
                            if norm_rest:
                                norm_rest.pop(0)()
                            if fillers is not None:
                                fillers(ci, kt, nkt)
                            if pend is not None:
                                _kt, _e, _lo = pend
                                for s in range(2):
                                    hb = 2 * p + s
                                    nc.tensor.matmul(
                                        o_ps[s][:, _lo:qcw],
                                        v_sb[_kt][:, hb * 65:(hb + 1) * 65],
                                        _e[:, s, _lo:qcw],
                                        start=(_kt == 0), stop=False,
                                        skip_group_check=True)
                            pend = (kt, e_v, lo)
                            if last and kt == nkt - 2:
                                # PV(kt-1) just issued; chunk columns
                                # [0, split) are final in PSUM
                                split, hookA, _ = stage
                                emit_norm(p, q0, 0, split, o_ps, defer=False,
                                          mm_bcast=True)
                                hookA()
                        _kt, _e, _lo = pend
                        for s in range(2):
                            hb = 2 * p + s
                            nc.tensor.matmul(
                                o_ps[s][:, _lo:qcw],
                                v_sb[_kt][:, hb * 65:(hb + 1) * 65],
                                _e[:, s, _lo:qcw],
                                start=False, stop=True,
                                skip_group_check=True)
                        if last:
                            split, _, hookB = stage
                            emit_norm(p, q0, split, qcw, o_ps, defer=False,
                                      mm_bcast=True)
                            hookB()
                        else:
                            emit_norm(p, q0, 0, qcw, o_ps)
                        yield ci
                    while norm_rest:
                        norm_rest.pop(0)()

                # ---- PE warmup + pair 0 front: j0 q/k chains ----
                with tc.tile_pool(name="ppsA", bufs=2, space="PSUM") as ppsA:
                    ps_q0 = ppsA.tile([128, 512], F32, tag="projA", name="ps_q0")
                    ps_k0 = ppsA.tile([128, 512], F32, tag="projA", name="ps_k0")
                    for i in range(NWARM):
                        nc.tensor.matmul(ps_q0, warm_t[:, 0:128], warm_t,
                                         start=True, stop=True,
                                         skip_group_check=True)
                    for c in range(8):
                        nc.tensor.matmul(
                            ps_q0, wq_v[:, c, 0:128], xt0[:, c, :],
                            start=(c == 0), stop=(c == 7))
                    nc.vector.tensor_scalar_add(
                        qT_sb[0][:, 0:512], ps_q0, bq_sb[:, 0:1])
                    # warmup bridges: dependency-free matmuls that keep the PE
                    # busy (HAM stays at full clock) across the DMA-paced
                    # waits for wk / the q,kT copies before chunk-0 scores
                    for i in range(3):
                        nc.tensor.matmul(ps_q0, warm_t[:, 0:128], warm_t,
                                         start=True, stop=True,
                                         skip_group_check=True)
                    for c in range(8):
                        nc.tensor.matmul(
                            ps_k0, wk_v[:, c, 0:128], xt0[:, c, :],
                            start=(c == 0), stop=(c == 7))
                    nc.vector.tensor_copy(kT_sb[0][:, 0:512], ps_k0)
                    for i in range(2):
                        nc.tensor.matmul(ps_k0, warm_t[:, 0:128], warm_t,
                                         start=True, stop=True,
                                         skip_group_check=True)
                with (
                    tc.tile_pool(name="sq", bufs=2, space="PSUM") as sqp,
                    tc.tile_pool(name="ops", bufs=1, space="PSUM") as opp,
                ):
                    with tc.tile_pool(name="pps", bufs=2, space="PSUM") as pps:
                        # pair-0 fillers, placed to match DMA arrivals:
                        # chunk 0: v0-1 + pair-0 j1 (x block 1 lands mid-chunk)
                        # chunk 1: v2-7 + pair-0 j2
                        # chunk 2: v8-11 + pair-0 j3
                        # chunk 3: v12-15 + pair-1 j0 (pair-1 j1-j3 moved into
                        #          pair-1's own slots — pair 0 runs PE-
                        #          saturated while pair 1 has idle slots)
                        def qkf(p_, j_):
                            return [lambda: qk_chain(p_, j_, "q", pps),
                                    lambda: qk_chain(p_, j_, "k", pps)]

                        def vf(a_, b_):
                            return [lambda t=t: v_chain(t, pps)
                                    for t in range(a_, b_)]

                        # v(t) must be emitted in or before the chunk whose PV
                        # consumes it (chunk ci's PV covers k-tiles < 4*(ci+1))
                        sched = {0: vf(0, 4) + qkf(0, 1),
                                 1: vf(4, 8) + qkf(0, 2),
                                 2: vf(8, 12) + qkf(0, 3),
                                 3: vf(12, 16) + qkf(1, 0)}
                        queues = [list(sched.get(ci, [])) for ci in range(4)]

                        def filler_pop(ci, kt, nkt):
                            q = queues[ci]
                            rem_slots = nkt - kt
                            while q and len(q) >= rem_slots:
                                q.pop(0)()
                            if q:
                                q.pop(0)()

                        for _ci in emit_pair(0, filler_pop):
                            while queues[_ci]:
                                queues[_ci].pop(0)()

                    if debug:
                        for p in range(2):
                            nc.sync.dma_start(out=dbg["qT"][:, p, :], in_=qT_sb[p])
                            nc.sync.dma_start(out=dbg["kT"][:, p, :], in_=kT_sb[p])
                        for t in range(NKT):
                            nc.sync.dma_start(out=dbg["vv"][:, t, :], in_=v_sb[t])

                    # ---- pair 1: output projection as fillers ----
                    with tc.tile_pool(name="fps", bufs=2, space="PSUM") as fps:
                        with tc.tile_pool(name="fsb", bufs=4) as fsb:
                            c_alt = [0]
                            fsb_tiles = {}

                            def c_unit(qt, oc_i, vec=None):
                                f_ps = fps.tile([128, 512], F32, tag="f", name="f_ps")
                                for p in range(2):
                                    nc.tensor.matmul(
                                        f_ps, outT_sb[p][:, qt * 128:(qt + 1) * 128],
                                        wo_sb[p][:, oc_i * 512:(oc_i + 1) * 512],
                                        start=(p == 0), stop=(p == 1))
                                if qt not in fsb_tiles:
                                    fsb_tiles[qt] = fsb.tile(
                                        [128, 1024], F16, tag="f", name=f"fsb{qt}")
                                half = fsb_tiles[qt][:, oc_i * 512:(oc_i + 1) * 512]
                                c_alt[0] ^= 1
                                use_vec = vec if vec is not None else c_alt[0]
                                if use_vec:
                                    nc.vector.tensor_copy(half, f_ps)
                                else:
                                    nc.scalar.activation(half, f_ps, AF.Identity)
                                if oc_i == 1:
                                    # one HWDGE DMA per 128-row output block,
                                    # alternating the two HWDGE rings
                                    eng = nc.sync if qt % 2 == 0 else nc.scalar
                                    eng.dma_start(
                                        out=y[qt * 128:(qt + 1) * 128, :],
                                        in_=fsb_tiles.pop(qt))

                            NCH = len(CHUNKS[1])
                            # pair-1's own j1-j3 projection chains run as
                            # early-chunk fillers here (chunk ci's scores only
                            # need q/kT cols up to (ci+1)*512, so j(ci+1)
                            # finishing inside chunk ci is in time).
                            # Queue items take a near_end hint: in a chunk's
                            # last two slots the out-proj cast goes to the
                            # scalar engine so the DVE queue is clear for the
                            # o_ps-draining copies at the boundary.
                            cqueues = [[] for _ in range(NCH)]
                            for ci in range(3):
                                cqueues[ci] += [
                                    lambda ne=False, j=ci + 1:
                                        qk_chain(1, j, "q", fps, tag="f"),
                                    lambda ne=False, j=ci + 1:
                                        qk_chain(1, j, "k", fps, tag="f")]

                            def c_pop(ci, kt, nkt):
                                # front-load just after the boundary (kt 1-2,
                                # not 0 — the deferred norm multiplies own the
                                # DVE at kt 0)
                                q = cqueues[ci]
                                ne = kt >= nkt - 2
                                rem_slots = nkt - kt
                                while q and len(q) >= rem_slots:
                                    q.pop(0)(ne)
                                for _ in range(2 if kt in (1, 2) else 1):
                                    if q:
                                        q.pop(0)(ne)

                            qt_ranges = [(0, 4), (4, 8), (8, 12)]

                            def tail_hook_a():
                                # qt 12-13 project while k-tiles 14/15 finish;
                                # all casts on DVE — a scalar-engine cast here
                                # would queue behind the last exps (FIFO)
                                for qt in (12, 13):
                                    c_unit(qt, 0, vec=True)
                                    c_unit(qt, 1, vec=True)

                            def tail_hook_b():
                                for qt in (14, 15):
                                    c_unit(qt, 0, vec=True)
                                    c_unit(qt, 1, vec=True)

                            for ci in emit_pair(1, c_pop,
                                                stage=(256, tail_hook_a,
                                                       tail_hook_b)):
                                if ci < len(qt_ranges):
                                    a, b = qt_ranges[ci]
                                    units = []
                                    for qt in range(a, b):
                                        for oc_i in range(2):
                                            units.append(
                                                lambda ne=False, qt=qt, oc_i=oc_i:
                                                c_unit(qt, oc_i,
                                                       vec=(False if ne else None)))
                                    cqueues[ci + 1].extend(units)
                            for q in cqueues:
                                while q:
                                    q.pop(0)(False)

            if debug:
                for p in range(2):
                    nc.sync.dma_start(out=dbg["oT"][:, p, :], in_=outT_sb[p])

    nc.compile()
    return nc


_cached = {}


def _get_nc(debug=False):
    key = bool(debug)
    if key not in _cached:
        _cached[key] = _build(debug)
    return _cached[key]


def _pack_pcw(a):
    # [(c p), w] -> [p, (c w)]  (the SBUF tile layout)
    cp, w = a.shape
    c = cp // 128
    return np.ascontiguousarray(
        a.reshape(c, 128, w).transpose(1, 0, 2).reshape(128, c * w))


def _prep_inputs(x, w_q, b_q, w_k, w_v):
    tri = np.triu(np.ones((128, 128), np.float32)).astype(NPF16)
    tri2 = np.concatenate([tri, tri], axis=1)
    wqT_f = np.ascontiguousarray(w_q.T).astype(NPF16)
    wkT_f = np.ascontiguousarray(w_k.T).astype(NPF16)
    wvT_f = np.ascontiguousarray(w_v.T).astype(NPF16)
    in_maps = []
    for core in range(N_CORES):
        b, hg = divmod(core, 4)
        cs = slice(hg * 256, (hg + 1) * 256)
        xm = x[b].T.astype(NPF16)  # [D, S]
        in_maps.append({
            "x0p": _pack_pcw(xm[:, 0:512]),
            "x1ap": _pack_pcw(xm[:, 512:1024]),
            "x1bp": _pack_pcw(xm[:, 1024:1536]),
            "x1cp": _pack_pcw(xm[:, 1536:2048]),
            "wqp": _pack_pcw(wqT_f[:, cs]),
            "wkp": _pack_pcw(wkT_f[:, cs]),
            "wvp": _pack_pcw(wvT_f[:, cs]),
            "bq2": np.ascontiguousarray(
                b_q[hg * 256:(hg + 1) * 256].reshape(2, 128).T.astype(np.float32)),
            "tri2": tri2,
        })
    return in_maps


def _numpy_reference(x, attention_mask, w_q, b_q, w_k, b_k, w_v, b_v, w_o, b_o):
    x = x.astype(np.float64)
    q = (x @ w_q.T + b_q).reshape(B, S, H, DK).transpose(0, 2, 1, 3)
    k = (x @ w_k.T + b_k).reshape(B, S, H, DK).transpose(0, 2, 1, 3)
    v = (x @ w_v.T + b_v).reshape(B, S, H, DK).transpose(0, 2, 1, 3)
    scores = np.einsum("bhqd,bhkd->bhqk", q, k, optimize=True) / np.sqrt(DK)
    causal = np.tril(np.ones((S, S), bool))
    mask = causal[None, None] & (attention_mask[:, None, None, :] != 0)
    scores = np.where(mask, scores, -np.inf)
    scores -= scores.max(-1, keepdims=True)
    e = np.exp(scores)
    attn = e / e.sum(-1, keepdims=True)
    out = np.einsum("bhqk,bhkd->bhqd", attn, v, optimize=True)
    out = out.transpose(0, 2, 1, 3).reshape(B, S, D)
    return (out @ w_o.T + b_o).astype(np.float32)


def kernel(x, attention_mask, w_q, b_q, w_k, b_k, w_v, b_v, w_o, b_o,
           _debug=False, _trace=False):
    x = np.asarray(x, np.float32)
    attention_mask = np.asarray(attention_mask)
    if not np.all(attention_mask != 0):
        return _numpy_reference(np.asarray(x), np.asarray(attention_mask),
                                *[np.asarray(a) for a in
                                  (w_q, b_q, w_k, b_k, w_v, b_v, w_o, b_o)])
    w_q, w_k, w_v, w_o = [np.asarray(w, np.float32) for w in (w_q, w_k, w_v, w_o)]
    b_q, b_k, b_v, b_o = [np.asarray(b, np.float32) for b in (b_q, b_k, b_v, b_o)]

    nc = _get_nc(_debug)
    in_maps = _prep_inputs(x, w_q, b_q, w_k, w_v)
    woT_f = np.ascontiguousarray(w_o.T).astype(NPF16)
    for core in range(N_CORES):
        hg = core % 4
        in_maps[core]["woT"] = np.ascontiguousarray(
            woT_f[hg * 256:(hg + 1) * 256, :])

    res = run_bass_kernel_spmd(nc, in_maps, list(range(N_CORES)), trace=_trace)
    const_row = (b_v @ w_o.T + b_o).astype(np.float32)
    y = np.zeros((B, S, D), np.float32)
    for core in range(N_CORES):
        b = core // 4
        y[b] += res.results[core]["y"].astype(np.float32)
    y += const_row
    if _debug or _trace:
        return y, res
    return y


# revision 39
# speedup vs baseline: 1.2587x; 1.0266x over previous
"""Multi-head causal self-attention (B=2, S=2048, D=1024, H=16) on 8 TRN2 cores.

Sharding: core = b*4 + hg  (b in {0,1} batch, hg in {0..3} head-group of 4 heads).
Per core: project qT/kT (pair-packed [128, S], fp16) and v ([S, 64] blocks, fp16),
compute transposed scores S^T = K Q^T per head (k on partitions, two heads per
k-tile via tile_position), exp on ScalarE (both heads in one strided activation,
fp16 out), causal diag masking via one fused strided tensor_mul against a
duplicated upper-tri matrix, PV matmul with a ones-column PREPENDED to V (col 0
of each 128-wide head block, so row 0 of the accumulator is the softmax sum on
partition 0 — the DVE reciprocal mis-executes on HW with a partition-offset
input) and the block zero-padded to 128 columns so the compiler's fast-weight-
load path (NumWeights==128) keeps PV LDWEIGHTS off the critical path.
Normalization per chunk: copy the value rows out of PSUM, reciprocal straight
from the PSUM sums row, then (deferred one-per-kt into the next chunk) a
broadcast of rec and a fused multiply+cast per head. Pair-0 norms broadcast on
the idle GpSimd Q7 ring (PE is ~98% busy there); pair-1 norms broadcast on the
tensor engine (ones[1,64].T @ rec16 — PE has slack there and the 1.8us Q7
launches otherwise sit in pair-1's critical path and re-throttle the PE).
Output projection: per 128-row block both 512-wide halves cast into one
[128,1024] SBUF tile (casts split across DVE and ACT), then a single HWDGE DMA
on the Sync ring per block. Host sums the 4 per-batch partials and adds
(b_v @ w_o.T + b_o); b_k is dropped (softmax is invariant to per-query
constants); b_q is applied on-device.
Startup: inputs stream on the Sync HWDGE ring (cheap RTL descriptor generation;
the gpsimd SWDGE DIRECT2D path costs ~1us of Q7 per trigger and serialized the
old startup) in exact need order with wq split so the first projection chain
starts after ~400KB; the x tail columns + w_o ride the gpsimd ring in dead
time. While the first inputs stream, the PE runs warmup matmuls on a memset
scratch tile so the HAM activity window is full (2.4 GHz) when real work lands.
All matmul operands are fp16 (same PE rate as bf16, 8x the mantissa);
accumulation is fp32 in PSUM.
"""

import numpy as np
import ml_dtypes

import concourse.bass as bass
import concourse.mybir as mybir
import concourse.tile as tile
from concourse import bacc
from concourse.bass_utils import run_bass_kernel_spmd

B, S, D, H, DK = 2, 2048, 1024, 16, 64
N_CORES = 8
F32 = mybir.dt.float32
F16 = mybir.dt.float16
NPF16 = np.float16
AF = mybir.ActivationFunctionType

CHUNKS = [
    [(0, 512), (512, 512), (1024, 512), (1536, 512)],
    [(0, 512), (512, 512), (1024, 512), (1536, 512)],
]

NWARM = 5  # PE warmup matmuls (N=512) during the startup DMA wait


def _build(debug=False):
    nc = bacc.Bacc("TRN2", target_bir_lowering=False, debug=False,
                   num_devices=N_CORES)
    # inputs are host-packed into the exact SBUF tile layouts ("p (c w)"),
    # so every DMA is a partition-contiguous slab (max SDMA efficiency)
    x0p = nc.dram_tensor("x0p", [128, 4096], F16, kind="ExternalInput").ap()
    x1ap = nc.dram_tensor("x1ap", [128, 4096], F16, kind="ExternalInput").ap()
    x1bp = nc.dram_tensor("x1bp", [128, 4096], F16, kind="ExternalInput").ap()
    x1cp = nc.dram_tensor("x1cp", [128, 4096], F16, kind="ExternalInput").ap()
    wqp = nc.dram_tensor("wqp", [128, 2048], F16, kind="ExternalInput").ap()
    wkp = nc.dram_tensor("wkp", [128, 2048], F16, kind="ExternalInput").ap()
    wvp = nc.dram_tensor("wvp", [128, 2048], F16, kind="ExternalInput").ap()
    woT = nc.dram_tensor("woT", [256, D], F16, kind="ExternalInput").ap()
    bq2 = nc.dram_tensor("bq2", [128, 2], F32, kind="ExternalInput").ap()
    tri2 = nc.dram_tensor("tri2", [128, 256], F16, kind="ExternalInput").ap()
    scr = nc.dram_tensor("scr", [2, 1024], F32, kind="Internal").ap()
    y = nc.dram_tensor("y", [S, D], F16, kind="ExternalOutput").ap()
    dbg = {}
    if debug:
        for nm, shp in [("qT", [128, 2, S]), ("kT", [128, 2, S]),
                        ("vv", [128, 16, 260]), ("oT", [128, 2, S])]:
            dbg[nm] = nc.dram_tensor(nm, shp, F16, kind="ExternalOutput").ap()

    NKT = S // 128   # k tiles

    with tile.TileContext(nc) as tc, \
            nc.allow_low_precision(reason="fp16 attention kernel"):
        with (
            tc.tile_pool(name="persist", bufs=1) as persist,
            tc.tile_pool(name="kqv", bufs=2) as kqv,
        ):
            qT_sb = [kqv.tile([128, S], F16, tag="qT", name=f"qT{p}") for p in range(2)]
            kT_sb = [kqv.tile([128, S], F16, tag="kT", name=f"kT{p}") for p in range(2)]
            v_sb = [persist.tile([128, 4 * 65], F16, tag=f"v{t}", name=f"v{t}")
                    for t in range(NKT)]
            outT_sb = [persist.tile([128, S], F16, tag=f"oT{p}", name=f"oTs{p}")
                       for p in range(2)]
            wo_sb = [persist.tile([128, D], F16, tag=f"wo{p}", name=f"wo{p}")
                     for p in range(2)]
            tri_sb = persist.tile([128, 256], F16, tag="tri")
            bq_sb = persist.tile([128, 2], F32, tag="bq")
            ones_sb = persist.tile([1, 64], F16, tag="ones1")
            tri_v = tri_sb.rearrange("p (c w) -> p c w", c=2)

            with (
                tc.tile_pool(name="xw", bufs=1) as xw,
                tc.tile_pool(name="ep", bufs=5) as ep,
                tc.tile_pool(name="rp", bufs=4) as rp,
            ):
                xt0_t = xw.tile([128, 8 * 512], F16, tag="x0", name="xt0")
                xt1a_t = xw.tile([128, 8 * 512], F16, tag="x1a", name="xt1a")
                xt1b_t = xw.tile([128, 8 * 512], F16, tag="x1b", name="xt1b")
                xt1c_t = xw.tile([128, 8 * 512], F16, tag="x1c", name="xt1c")
                wq_t = xw.tile([128, 8 * 256], F16, tag="wq", name="wqs")
                wk_t = xw.tile([128, 8 * 256], F16, tag="wk", name="wks")
                wv_t = xw.tile([128, 8 * 256], F16, tag="wv", name="wvs")
                warm_t = xw.tile([128, 512], F16, tag="warm", name="warm")
                xt0 = xt0_t.rearrange("p (c w) -> p c w", c=8)
                xt1v = [t.rearrange("p (c w) -> p c w", c=8)
                        for t in (xt1a_t, xt1b_t, xt1c_t)]
                wq_v = wq_t.rearrange("p (c w) -> p c w", c=8)
                wk_v = wk_t.rearrange("p (c w) -> p c w", c=8)
                wv_v = wv_t.rearrange("p (c w) -> p c w", c=8)

                # Two HWDGE rings in parallel (each sustains only ~half the
                # HBM wire), both in need order: x-side on sync, weight-side
                # on scalar; tri alone on the gpsimd SWDGE ring. Every
                # transfer is a contiguous slab of a host-packed tensor.
                nc.sync.dma_start(out=xt0_t[:, 0:1024], in_=x0p[:, 0:1024])
                nc.sync.dma_start(out=xt0_t[:, 1024:2048], in_=x0p[:, 1024:2048])
                nc.sync.dma_start(out=xt0_t[:, 2048:4096], in_=x0p[:, 2048:4096])
                nc.sync.dma_start(out=xt1a_t, in_=x1ap)
                nc.sync.dma_start(out=xt1b_t, in_=x1bp)
                nc.sync.dma_start(out=xt1c_t, in_=x1cp)
                nc.scalar.dma_start(out=bq_sb, in_=bq2)
                nc.scalar.dma_start(out=wq_t[:, 0:512], in_=wqp[:, 0:512])
                nc.scalar.dma_start(out=wq_t[:, 512:2048], in_=wqp[:, 512:2048])
                nc.scalar.dma_start(out=wk_t, in_=wkp)
                nc.scalar.dma_start(out=wv_t, in_=wvp)
                for p in range(2):
                    nc.scalar.dma_start(out=wo_sb[p], in_=woT[p * 128:(p + 1) * 128, :])
                nc.gpsimd.dma_start(out=tri_sb, in_=tri2)

                nc.vector.memset(ones_sb, 1.0)
                nc.vector.memset(warm_t, 0.125)
                # preload the exp table set (~2.7us ACT_TABLE_LOAD) during the
                # startup DMA wait instead of before the first real exp
                dummy_e = ep.tile([1, 64], F16, tag="e", name="dummy_e")
                nc.scalar.activation(dummy_e, warm_t[0:1, 0:64], AF.Exp,
                                     scale=0.125)

                def xcols(c, a, b):
                    blk = a // 512
                    assert b <= (blk + 1) * 512
                    if blk == 0:
                        return xt0[:, c, a:b]
                    t = xt1v[blk - 1]
                    return t[:, c, a - blk * 512:b - blk * 512]

                def qk_chain(p, j, which, pool, tag="proj"):
                    ps = pool.tile([128, 512], F32, tag=tag, name="ps")
                    w_v_ = wq_v if which == "q" else wk_v
                    for c in range(8):
                        nc.tensor.matmul(
                            ps, w_v_[:, c, p * 128:(p + 1) * 128],
                            xcols(c, j * 512, (j + 1) * 512),
                            start=(c == 0), stop=(c == 7))
                    if which == "q":
                        nc.vector.tensor_scalar_add(
                            qT_sb[p][:, j * 512:(j + 1) * 512], ps,
                            bq_sb[:, p:p + 1])
                    else:
                        nc.vector.tensor_copy(
                            kT_sb[p][:, j * 512:(j + 1) * 512], ps)

                def v_chain(t, pool):
                    ps_v = pool.tile([128, 256], F32, tag="proj", name="ps_v")
                    for c in range(8):
                        nc.tensor.matmul(
                            ps_v, xcols(c, t * 128, (t + 1) * 128), wv_v[:, c, :],
                            start=(c == 0), stop=(c == 7))
                    v_view = v_sb[t].rearrange("p (h w) -> p h w", w=65)
                    nc.vector.memset(v_view[:, :, 64:65], 1.0)
                    nc.vector.tensor_copy(
                        v_view[:, :, 0:64],
                        ps_v.rearrange("p (h w) -> p h w", w=64))

                norm_rest = []

                def emit_norm(p_, q0_, a_, b_, o_ps_, defer=True,
                              mm_bcast=False):
                    # copy values + the sums row out of PSUM for chunk columns
                    # [a, b); sums staged to partition 0 (PSUM reads must
                    # start partition-aligned, and the DVE reciprocal
                    # mis-executes on HW with a partition-offset input).
                    # The broadcast launches HERE (Q7 is idle; its ~1.8us
                    # latency hides across the chunk boundary) — only the
                    # final multiplies defer into the next chunk's slots. The
                    # staged last-chunk norms broadcast on the tensor engine
                    # instead (zero added latency before the output tail).
                    w_ = b_ - a_
                    rec = rp.tile([1, 1024], F32, tag="rec", name="recip")
                    sums = rp.tile([1, 1024], F32, tag="sums", name="sums")
                    ocs = []
                    for s in range(2):
                        oc = rp.tile([64, 512], F32, tag=f"oc{s}", name="o_cp")
                        nc.vector.tensor_copy(oc[:, 0:w_], o_ps_[s][0:64, a_:b_])
                        nc.vector.tensor_copy(
                            sums[:, s * 512:s * 512 + w_],
                            o_ps_[s][64:65, a_:b_])
                        ocs.append(oc)
                    for s in range(2):
                        nc.vector.reciprocal_approx_fast(
                            out=rec[:, s * 512:s * 512 + w_],
                            in_=sums[:, s * 512:s * 512 + w_])
                    if mm_bcast:
                        rec16 = rp.tile([1, 1024], F16, tag="rec16", name="r16")
                        nc.vector.tensor_copy(rec16, rec)
                        bc_list = []
                        for s in range(2):
                            bc_ps = fps.tile([128, 512], F32, tag="f", name="bcp")
                            nc.tensor.matmul(
                                bc_ps[0:64, :], ones_sb,
                                rec16[:, s * 512:(s + 1) * 512],
                                start=True, stop=True, skip_group_check=True)
                            bc_list.append(bc_ps[0:64, :])
                    else:
                        bcs = rp.tile([64, 1024], F32, tag="bc", name="bc")
                        nc.gpsimd.partition_broadcast(bcs, rec)
                        bc_list = [bcs[:, s * 512:(s + 1) * 512] for s in range(2)]
                    if defer:
                        norm_rest.append(
                            lambda: finish_norm(p_, q0_ + a_, w_, bc_list, ocs))
                    else:
                        finish_norm(p_, q0_ + a_, w_, bc_list, ocs)

                # NOTE: the multiplies must stay on the DVE — running them on
                # GpSimd interleaves Multiply with PartitionBroadcast and the
                # Q7 LIBRARY_RELOAD churn between op types costs multiple us
                # of hidden latency per switch (measured: 17.9us stalls).
                def finish_norm(p_, qa_, w_, bc_list, ocs):
                    for s in range(2):
                        nc.vector.tensor_mul(
                            outT_sb[p_][s * 64:(s + 1) * 64, qa_:qa_ + w_],
                            ocs[s][:, 0:w_],
                            bc_list[s][:, 0:w_])

                def emit_pair(p, fillers, stage=None):
                    # stage: (split_col, hookA, hookB) applied to the LAST
                    # chunk: columns [0, split) normalize right after their
                    # final PV (two k-tiles early), so most of the output
                    # projection tail overlaps the chunk's trailing k-tiles.
                    nchunks = len(CHUNKS[p])
                    for ci, (q0, qcw) in enumerate(CHUNKS[p]):
                        nkt = (q0 + qcw) // 128
                        last = stage is not None and ci == nchunks - 1
                        o_ps = [opp.tile([65, 512], F32, tag=f"o{s}", name=f"ops{s}")
                                for s in range(2)]
                        pend = None
                        for kt in range(nkt):
                            o = kt * 128 - q0
                            diag = o >= 0
                            lo = o if diag else 0
                            s_ab = sqp.tile([128, 1024], F32, tag="sq", name="s_ab")
                            s_v = s_ab.rearrange("p (c w) -> p c w", c=2)
                            for s in range(2):
                                nc.tensor.matmul(
                                    s_v[:, s, lo:qcw],
                                    kT_sb[p][s * 64:(s + 1) * 64,
                                             kt * 128:(kt + 1) * 128],
                                    qT_sb[p][s * 64:(s + 1) * 64,
                                             q0 + lo:q0 + qcw],
                                    start=True, stop=True,
                                    tile_position=(s * 64, 0),
                                    skip_group_check=True)
                            e_ab = ep.tile([128, 1024], F16, tag="e", name="e_ab")
                            e_v = e_ab.rearrange("p (c w) -> p c w", c=2)
                            nc.scalar.activation(
                                e_v[:, :, lo:qcw], s_v[:, :, lo:qcw],
                                AF.Exp, scale=0.125)
                            if diag:
                                # two contiguous 1-D multiplies: the strided
                                # [p, 2, 128] form runs at half the DVE rate
                                for s in range(2):
                                    nc.vector.tensor_mul(
                                        e_ab[:, s * 512 + o:s * 512 + o + 128],
                                        e_ab[:, s * 512 + o:s * 512 + o + 128],
                                        tri_sb[:, s * 128:(s + 1) * 128])
                            if norm_rest:
                                norm_rest.pop(0)()
                            if fillers is not None:
                                fillers(ci, kt, nkt)
                            if pend is not None:
                                _kt, _e, _lo = pend
                                for s in range(2):
                                    hb = 2 * p + s
                                    nc.tensor.matmul(
                                        o_ps[s][:, _lo:qcw],
                                        v_sb[_kt][:, hb * 65:(hb + 1) * 65],
                                        _e[:, s, _lo:qcw],
                                        start=(_kt == 0), stop=False,
                                        skip_group_check=True)
                            pend = (kt, e_v, lo)
                            if last and kt == nkt - 2:
                                # PV(kt-1) just issued; chunk columns
                                # [0, split) are final in PSUM
                                split, hookA, _ = stage
                                emit_norm(p, q0, 0, split, o_ps, defer=False,
                                          mm_bcast=True)
                                hookA()
                        _kt, _e, _lo = pend
                        for s in range(2):
                            hb = 2 * p + s
                            nc.tensor.matmul(
                                o_ps[s][:, _lo:qcw],
                                v_sb[_kt][:, hb * 65:(hb + 1) * 65],
                                _e[:, s, _lo:qcw],
                                start=False, stop=True,
                                skip_group_check=True)
                        if last:
                            split, _, hookB = stage
                            emit_norm(p, q0, split, qcw, o_ps, defer=False,
                                      mm_bcast=True)
                            hookB()
                        else:
                            emit_norm(p, q0, 0, qcw, o_ps)
                        yield ci
                    while norm_rest:
                        norm_rest.pop(0)()

                # ---- PE warmup + pair 0 front: j0 q/k chains ----
                with tc.tile_pool(name="ppsA", bufs=2, space="PSUM") as ppsA:
                    ps_q0 = ppsA.tile([128, 512], F32, tag="projA", name="ps_q0")
                    ps_k0 = ppsA.tile([128, 512], F32, tag="projA", name="ps_k0")
                    for i in range(NWARM):
                        nc.tensor.matmul(ps_q0, warm_t[:, 0:128], warm_t,
                                         start=True, stop=True,
                                         skip_group_check=True)
                    for c in range(8):
                        nc.tensor.matmul(
                            ps_q0, wq_v[:, c, 0:128], xt0[:, c, :],
                            start=(c == 0), stop=(c == 7))
                    nc.vector.tensor_scalar_add(
                        qT_sb[0][:, 0:512], ps_q0, bq_sb[:, 0:1])
                    # warmup bridges: dependency-free matmuls that keep the PE
                    # busy (HAM stays at full clock) across the DMA-paced
                    # waits for wk / the q,kT copies before chunk-0 scores
                    for i in range(3):
                        nc.tensor.matmul(ps_q0, warm_t[:, 0:128], warm_t,
                                         start=True, stop=True,
                                         skip_group_check=True)
                    for c in range(8):
                        nc.tensor.matmul(
                            ps_k0, wk_v[:, c, 0:128], xt0[:, c, :],
                            start=(c == 0), stop=(c == 7))
                    nc.vector.tensor_copy(kT_sb[0][:, 0:512], ps_k0)
                    for i in range(2):
                        nc.tensor.matmul(ps_k0, warm_t[:, 0:128], warm_t,
                                         start=True, stop=True,
                                         skip_group_check=True)
                with (
                    tc.tile_pool(name="sq", bufs=2, space="PSUM") as sqp,
                    tc.tile_pool(name="ops", bufs=1, space="PSUM") as opp,
                ):
                    with tc.tile_pool(name="pps", bufs=2, space="PSUM") as pps:
                        # pair-0 fillers, placed to match DMA arrivals:
                        # chunk 0: v0-1 + pair-0 j1 (x block 1 lands mid-chunk)
                        # chunk 1: v2-7 + pair-0 j2
                        # chunk 2: v8-11 + pair-0 j3
                        # chunk 3: v12-15 + pair-1 j0 (pair-1 j1-j3 moved into
                        #          pair-1's own slots — pair 0 runs PE-
                        #          saturated while pair 1 has idle slots)
                        def qkf(p_, j_):
                            return [lambda: qk_chain(p_, j_, "q", pps),
                                    lambda: qk_chain(p_, j_, "k", pps)]

                        def vf(a_, b_):
                            return [lambda t=t: v_chain(t, pps)
                                    for t in range(a_, b_)]

                        # v(t) must be emitted in or before the chunk whose PV
                        # consumes it (chunk ci's PV covers k-tiles < 4*(ci+1))
                        sched = {0: vf(0, 4) + qkf(0, 1),
                                 1: vf(4, 8) + qkf(0, 2),
                                 2: vf(8, 12) + qkf(0, 3),
                                 3: vf(12, 16) + qkf(1, 0)}
                        queues = [list(sched.get(ci, [])) for ci in range(4)]

                        def filler_pop(ci, kt, nkt):
                            q = queues[ci]
                            rem_slots = nkt - kt
                            while q and len(q) >= rem_slots:
                                q.pop(0)()
                            if q:
                                q.pop(0)()

                        for _ci in emit_pair(0, filler_pop):
                            while queues[_ci]:
                                queues[_ci].pop(0)()

                    if debug:
                        for p in range(2):
                            nc.sync.dma_start(out=dbg["qT"][:, p, :], in_=qT_sb[p])
                            nc.sync.dma_start(out=dbg["kT"][:, p, :], in_=kT_sb[p])
                        for t in range(NKT):
                            nc.sync.dma_start(out=dbg["vv"][:, t, :], in_=v_sb[t])

                    # ---- pair 1: output projection as fillers ----
                    with tc.tile_pool(name="fps", bufs=2, space="PSUM") as fps:
                        with tc.tile_pool(name="fsb", bufs=4) as fsb:
                            c_alt = [0]
                            fsb_tiles = {}

                            def c_unit(qt, oc_i, vec=None, split_dma=False):
                                f_ps = fps.tile([128, 512], F32, tag="f", name="f_ps")
                                for p in range(2):
                                    nc.tensor.matmul(
                                        f_ps, outT_sb[p][:, qt * 128:(qt + 1) * 128],
                                        wo_sb[p][:, oc_i * 512:(oc_i + 1) * 512],
                                        start=(p == 0), stop=(p == 1))
                                if qt not in fsb_tiles:
                                    fsb_tiles[qt] = fsb.tile(
                                        [128, 1024], F16, tag="f", name=f"fsb{qt}")
                                half = fsb_tiles[qt][:, oc_i * 512:(oc_i + 1) * 512]
                                c_alt[0] ^= 1
                                use_vec = vec if vec is not None else c_alt[0]
                                if use_vec:
                                    nc.vector.tensor_copy(half, f_ps)
                                else:
                                    nc.scalar.activation(half, f_ps, AF.Identity)
                                if split_dma:
                                    # tail: ship each 512-half as soon as its
                                    # cast lands so the final transfer is half
                                    # the size
                                    eng = nc.sync if (2 * qt + oc_i) % 2 == 0 \
                                        else nc.scalar
                                    eng.dma_start(
                                        out=y[qt * 128:(qt + 1) * 128,
                                              oc_i * 512:(oc_i + 1) * 512],
                                        in_=half)
                                    if oc_i == 1:
                                        fsb_tiles.pop(qt)
                                elif oc_i == 1:
                                    # one HWDGE DMA per 128-row output block,
                                    # alternating the two HWDGE rings
                                    eng = nc.sync if qt % 2 == 0 else nc.scalar
                                    eng.dma_start(
                                        out=y[qt * 128:(qt + 1) * 128, :],
                                        in_=fsb_tiles.pop(qt))

                            NCH = len(CHUNKS[1])
                            # pair-1's own j1-j3 projection chains run as
                            # early-chunk fillers here (chunk ci's scores only
                            # need q/kT cols up to (ci+1)*512, so j(ci+1)
                            # finishing inside chunk ci is in time).
                            # Queue items take a near_end hint: in a chunk's
                            # last two slots the out-proj cast goes to the
                            # scalar engine so the DVE queue is clear for the
                            # o_ps-draining copies at the boundary.
                            cqueues = [[] for _ in range(NCH)]
                            for ci in range(3):
                                cqueues[ci] += [
                                    lambda ne=False, j=ci + 1:
                                        qk_chain(1, j, "q", fps, tag="f"),
                                    lambda ne=False, j=ci + 1:
                                        qk_chain(1, j, "k", fps, tag="f")]

                            def c_pop(ci, kt, nkt):
                                # front-load just after the boundary (kt 1-2,
                                # not 0 — the deferred norm multiplies own the
                                # DVE at kt 0)
                                q = cqueues[ci]
                                ne = kt >= nkt - 2
                                rem_slots = nkt - kt
                                while q and len(q) >= rem_slots:
                                    q.pop(0)(ne)
                                for _ in range(2 if kt in (1, 2) else 1):
                                    if q:
                                        q.pop(0)(ne)

                            qt_ranges = [(0, 4), (4, 8), (8, 12)]

                            def tail_hook_a():
                                # qt 12-13 project while k-tiles 14/15 finish;
                                # all casts on DVE — a scalar-engine cast here
                                # would queue behind the last exps (FIFO)
                                for qt in (12, 13):
                                    c_unit(qt, 0, vec=True, split_dma=True)
                                    c_unit(qt, 1, vec=True, split_dma=True)

                            def tail_hook_b():
                                for qt in (14, 15):
                                    c_unit(qt, 0, vec=True, split_dma=True)
                                    c_unit(qt, 1, vec=True, split_dma=True)

                            for ci in emit_pair(1, c_pop,
                                                stage=(256, tail_hook_a,
                                                       tail_hook_b)):
                                if ci < len(qt_ranges):
                                    a, b = qt_ranges[ci]
                                    units = []
                                    for qt in range(a, b):
                                        for oc_i in range(2):
                                            units.append(
                                                lambda ne=False, qt=qt, oc_i=oc_i:
                                                c_unit(qt, oc_i,
                                                       vec=(False if ne else None)))
                                    cqueues[ci + 1].extend(units)
                            for q in cqueues:
                                while q:
                                    q.pop(0)(False)

            if debug:
                for p in range(2):
                    nc.sync.dma_start(out=dbg["oT"][:, p, :], in_=outT_sb[p])

    nc.compile()
    return nc


_cached = {}


def _get_nc(debug=False):
    key = bool(debug)
    if key not in _cached:
        _cached[key] = _build(debug)
    return _cached[key]


def _pack_pcw(a):
    # [(c p), w] -> [p, (c w)]  (the SBUF tile layout)
    cp, w = a.shape
    c = cp // 128
    return np.ascontiguousarray(
        a.reshape(c, 128, w).transpose(1, 0, 2).reshape(128, c * w))


def _prep_inputs(x, w_q, b_q, w_k, w_v):
    tri = np.triu(np.ones((128, 128), np.float32)).astype(NPF16)
    tri2 = np.concatenate([tri, tri], axis=1)
    wqT_f = np.ascontiguousarray(w_q.T).astype(NPF16)
    wkT_f = np.ascontiguousarray(w_k.T).astype(NPF16)
    wvT_f = np.ascontiguousarray(w_v.T).astype(NPF16)
    in_maps = []
    for core in range(N_CORES):
        b, hg = divmod(core, 4)
        cs = slice(hg * 256, (hg + 1) * 256)
        xm = x[b].T.astype(NPF16)  # [D, S]
        in_maps.append({
            "x0p": _pack_pcw(xm[:, 0:512]),
            "x1ap": _pack_pcw(xm[:, 512:1024]),
            "x1bp": _pack_pcw(xm[:, 1024:1536]),
            "x1cp": _pack_pcw(xm[:, 1536:2048]),
            "wqp": _pack_pcw(wqT_f[:, cs]),
            "wkp": _pack_pcw(wkT_f[:, cs]),
            "wvp": _pack_pcw(wvT_f[:, cs]),
            "bq2": np.ascontiguousarray(
                b_q[hg * 256:(hg + 1) * 256].reshape(2, 128).T.astype(np.float32)),
            "tri2": tri2,
        })
    return in_maps


def _numpy_reference(x, attention_mask, w_q, b_q, w_k, b_k, w_v, b_v, w_o, b_o):
    x = x.astype(np.float64)
    q = (x @ w_q.T + b_q).reshape(B, S, H, DK).transpose(0, 2, 1, 3)
    k = (x @ w_k.T + b_k).reshape(B, S, H, DK).transpose(0, 2, 1, 3)
    v = (x @ w_v.T + b_v).reshape(B, S, H, DK).transpose(0, 2, 1, 3)
    scores = np.einsum("bhqd,bhkd->bhqk", q, k, optimize=True) / np.sqrt(DK)
    causal = np.tril(np.ones((S, S), bool))
    mask = causal[None, None] & (attention_mask[:, None, None, :] != 0)
    scores = np.where(mask, scores, -np.inf)
    scores -= scores.max(-1, keepdims=True)
    e = np.exp(scores)
    attn = e / e.sum(-1, keepdims=True)
    out = np.einsum("bhqk,bhkd->bhqd", attn, v, optimize=True)
    out = out.transpose(0, 2, 1, 3).reshape(B, S, D)
    return (out @ w_o.T + b_o).astype(np.float32)


def kernel(x, attention_mask, w_q, b_q, w_k, b_k, w_v, b_v, w_o, b_o,
           _debug=False, _trace=False):
    x = np.asarray(x, np.float32)
    attention_mask = np.asarray(attention_mask)
    if not np.all(attention_mask != 0):
        return _numpy_reference(np.asarray(x), np.asarray(attention_mask),
                                *[np.asarray(a) for a in
                                  (w_q, b_q, w_k, b_k, w_v, b_v, w_o, b_o)])
    w_q, w_k, w_v, w_o = [np.asarray(w, np.float32) for w in (w_q, w_k, w_v, w_o)]
    b_q, b_k, b_v, b_o = [np.asarray(b, np.float32) for b in (b_q, b_k, b_v, b_o)]

    nc = _get_nc(_debug)
    in_maps = _prep_inputs(x, w_q, b_q, w_k, w_v)
    woT_f = np.ascontiguousarray(w_o.T).astype(NPF16)
    for core in range(N_CORES):
        hg = core % 4
        in_maps[core]["woT"] = np.ascontiguousarray(
            woT_f[hg * 256:(hg + 1) * 256, :])

    res = run_bass_kernel_spmd(nc, in_maps, list(range(N_CORES)), trace=_trace)
    const_row = (b_v @ w_o.T + b_o).astype(np.float32)
    y = np.zeros((B, S, D), np.float32)
    for core in range(N_CORES):
        b = core // 4
        y[b] += res.results[core]["y"].astype(np.float32)
    y += const_row
    if _debug or _trace:
        return y, res
    return y
